# revision 13
# baseline (speedup 1.0000x reference)
"""Trainium2 Bass kernel for the MFPA attention module.

Reference computation (per batch b, with N = H*W = 4096 spatial sites):
    q = Wq @ x_RGB + bq            (CQK=16 channels)
    k = Wk @ x    + bk
    v = Wv @ x    + bv             (C=64 channels)
    energy[i,j] = q_i . k_j
    att = softmax(energy, axis=j)
    out[c,i] = sum_j v[c,j] att[i,j]
    y = lam * out + x

Device strategy (8 NeuronCores): data-parallel over batch (4) x query-row
halves (2).  Each core holds x[b] fully (for K/V and the residual) and its
2048-row query slice, and computes a flash-style streaming softmax so the
4096x4096 energy matrix never leaves PSUM/SBUF.

Host-side weight folding (softmax is shift-invariant, so bk drops out):
    energy[i,j] = (M^T xr_i + bqk) . xf_j    with  M = Wq^T Wk, bqk = Wk^T bq

v2 pipeline notes:
  * exp of the energy is split across BOTH the Scalar (true Exp -> fp8e4)
    and Vector engines (exponent-stuffing: byte = clamp(K8*e + 40) is the
    fp8e4 bit pattern of exp(e)/4, computed by one tensor_scalar).  A
    constant 40/K8 is folded into the energy via the ones row of xf_aug
    (extra qk channel), so the DVE op needs only (mult, max).
  * attention weights p and v are fp8e4; the PV matmul runs DoubleRow
    (256-deep contraction), halving tensor-engine streaming time.
  * the 1/rowsum uses the fast custom-DVE reciprocal; lam and a fp8
    range scale S are folded into wv on the host.
  * residual is taken from the bf16 xf tile (xres input dropped).
  * a short burst of dummy matmuls warms the PE HAM clock-gate while the
    input DMAs stream.
"""

import ml_dtypes
import numpy as np

import concourse.bass as bass
import concourse.mybir as mybir
import concourse.tile as tile_mod
from concourse.vector_clock import ScopedClock

B, C, HH, WW = 4, 64, 64, 64
N = HH * WW          # 4096 spatial sites
NI = N // 2          # query rows per core
CHUNK = 512          # query rows processed per main-loop iteration
NCHUNK = NI // CHUNK
JBLK = 128           # key/value block (PSUM partition dim)
NJP = N // (2 * JBLK)  # 16 j-pair groups (256 keys each)
NCORES = 8
NWARM = 10           # HAM warm-up matmuls

F32 = mybir.dt.float32
F32R = mybir.dt.float32r
BF16 = mybir.dt.bfloat16
F8 = mybir.dt.float8e4
U8 = mybir.dt.uint8

K8 = 8.0 / float(np.log(2.0))       # 11.5416  (fp8e4 octave = 8 bytes)
BIAS_B = 40.0                       # byte bias; fp8e4 exp bias is 56 -> p = exp(e)/4
BIAS_ROW = BIAS_B / K8              # energy offset carried by the qk ones-channel
ACT_BIAS = -(BIAS_ROW + float(2.0 * np.log(2.0)))   # Exp(e' + ACT_BIAS) = exp(e)/4
VSCALE = 16.0                       # fp8 range scale on v (cancels in p@v / p@1)
# exp on the vector engine for these j-pair groups (5 of 16; DVE also
# carries the reciprocal + epilogue), scalar takes 11
DVE_GROUPS = frozenset((2, 5, 8, 11, 14))


def _patched_drain_and_barrier(self, tick_clock, wait_clock):
    # The walrus build in this container rejects instructions with more than
    # one sync-wait command ("Too many sync wait commands" on the Tile tail
    # drain).  Split the aggregated drain into one drain per semaphore wait.
    nc = self.nc
    drain_inst = nc.sync.drain()
    wait_clock.add_sem_waits(
        drain_inst.ins, ScopedClock({None: tick_clock.global_clock})
    )
    inst = drain_inst.ins
    si = inst.sync_info
    waits = list(si.on_wait or []) if si else []
    if len(waits) > 1:
        si.on_wait = waits[:1]
        for w in waits[1:]:
            extra = nc.sync.drain()
            extra.ins.sync_info = mybir.SyncInfo(on_wait=[w], on_update=[])
    nc.all_engine_barrier()
    popped = nc._tile_sem_poison_stack.pop()
    assert popped is self._sem_poison
    nc.clear_and_free_semaphores(list(self.sems.allocated().values()))
    nc.all_engine_barrier()


tile_mod.TileContext._drain_and_barrier = _patched_drain_and_barrier


def _split_multi_waits(nc):
    # This walrus build accepts at most one sync-wait command per TPB
    # instruction.  Hoist extra waits onto engine NoOps placed just before
    # the instruction (engine executes in order, so semantics are kept).
    for blk in nc.m.functions[0].blocks:
        insts = list(blk.instructions)
        out = []
        changed = False
        for inst in insts:
            si = inst.sync_info
            if si is not None and si.on_wait and len(si.on_wait) > 1:
                waits = list(si.on_wait)
                si.on_wait = waits[-1:]
                for w in waits[:-1]:
                    nop = mybir.InstNoOp(name=nc.get_next_instruction_name())
                    nop.engine = inst.engine
                    nop.sync_info = mybir.SyncInfo(on_wait=[w], on_update=[])
                    out.append(nop)
                changed = True
            out.append(inst)
        if changed:
            blk.instructions = out


def build_bass(split_waits=True):
    nc = bass.Bass()
    xf = nc.declare_dram_parameter("xf", [C + 1, N], BF16, isOutput=False)
    xq = nc.declare_dram_parameter("xq", [C, NI], BF16, isOutput=False)
    # consts blob: cols 0:64 = M, col 64 = zero, col 65 = bias, cols 66:132 = wv_aug
    cb = nc.declare_dram_parameter("cb", [C + 1, 132], BF16, isOutput=False)
    onesv = nc.declare_dram_parameter("onesv", [1, C], F32R, isOutput=False)
    y = nc.declare_dram_parameter("y", [C, NI], F32, isOutput=True)

    with tile_mod.TileContext(nc) as tc:
        with (
            tc.tile_pool(name="singles", bufs=1) as singles,
            tc.tile_pool(name="ppool", bufs=4) as ppool,
            tc.tile_pool(name="ypool", bufs=3) as ypool,
            tc.tile_pool(name="lpool", bufs=2) as lpool,
            tc.tile_pool(name="rpool", bufs=2) as rpool,
            tc.tile_pool(name="ps_et", bufs=2, space="PSUM") as ps_et,
            tc.tile_pool(name="ps_pv", bufs=2, space="PSUM") as ps_pv,
            tc.tile_pool(name="ps_misc", bufs=2, space="PSUM") as ps_misc,
        ):
            # ---- tiny SBUF constants (no DMA) -----------------------------
            wsb = singles.tile([128, CHUNK], BF16)
            nc.vector.memset(wsb, 0.0)
            ones_sb = singles.tile([1, C], F32R)
            nc.gpsimd.dma_start(out=ones_sb, in_=onesv[:, :])
            actb_sb = singles.tile([128, 1], F32)
            nc.vector.memset(actb_sb, ACT_BIAS)

            # ---- input DMAs: one trigger per tensor/half ------------------
            xq_sb = singles.tile([C, NI], BF16)
            nc.sync.dma_start(out=xq_sb, in_=xq[:, :])
            cb_sb = singles.tile([C + 1, 132], BF16)
            nc.gpsimd.dma_start(out=cb_sb, in_=cb[:, :])
            xf_sb = singles.tile([C + 1, N], BF16)
            nc.gpsimd.dma_start(out=xf_sb[:, 0:NI], in_=xf[:, 0:NI])
            nc.sync.dma_start(out=xf_sb[:, NI:N], in_=xf[:, NI:N])
            # xf arrives column-permuted so this core's own query half sits
            # at columns 0:NI (softmax is j-permutation invariant); row C is
            # all-ones: it feeds the V bias row, the qk energy-bias channel
            # and the fp8 rowsum column.

            # ---- HAM warm-up: keep the PE busy while DMAs stream ----------
            for _ in range(NWARM):
                wp = ps_misc.tile([128, CHUNK], F32, tag="misc")
                nc.tensor.matmul(
                    out=wp, lhsT=wsb[:, 0:128], rhs=wsb, start=True, stop=True
                )

            bqk_sb = singles.tile([C + 1, 1], F32)
            nc.vector.tensor_copy(bqk_sb, cb_sb[0 : C + 1, 65:66])

            # ---- Q.K preparation for all chunks ---------------------------
            # qs row C is 0 (M blob column 64..65 region row C is zero), and
            # the bias vector carries BIAS_ROW there, so the Identity
            # activation fills the whole (C+1)-row qk tile in one op.
            qk_sbs = []
            for ic in range(NCHUNK):
                isl = slice(ic * CHUNK, (ic + 1) * CHUNK)
                qs = ps_misc.tile([C + 1, CHUNK], F32, tag="misc")
                nc.tensor.matmul(
                    out=qs, lhsT=cb_sb[0:C, 0:65], rhs=xq_sb[:, isl],
                    start=True, stop=True,
                )
                qk_sb = singles.tile([C + 1, CHUNK], BF16, name=f"qk_sb{ic}")
                nc.scalar.activation(
                    out=qk_sb, in_=qs,
                    func=mybir.ActivationFunctionType.Identity,
                    bias=bqk_sb[:, 0:1], scale=1.0,
                )
                qk_sbs.append(qk_sb)

            # ---- V preparation: fp8 v_pair[jp, pair, t, c] ----------------
            # pair-dim byte step must be 16-aligned for DoubleRow ldweights -> pad 66 to 80
            v_sb = singles.tile([JBLK, NJP, 2, 80], F8)
            for m4 in range(N // (4 * JBLK)):
                vp = ps_misc.tile([JBLK, 2, 2, 66], F32, tag="misc")
                for k in range(4):
                    jb = 4 * m4 + k
                    nc.tensor.matmul(
                        out=vp[:, k // 2, k % 2, :],
                        lhsT=xf_sb[:, jb * JBLK : (jb + 1) * JBLK],
                        rhs=cb_sb[:, 66:132],
                        start=True, stop=True,
                    )
                nc.scalar.activation(
                    out=v_sb[:, 2 * m4 : 2 * m4 + 2, :, 0:66], in_=vp,
                    func=mybir.ActivationFunctionType.Copy,
                )

            # ---- main loop over query chunks ------------------------------
            for ic in range(NCHUNK):
                isl = slice(ic * CHUNK, (ic + 1) * CHUNK)
                qk_sb = qk_sbs[ic]
                pv = ps_pv.tile([66, CHUNK], F32)
                pend = None
                for g in range(NJP):
                    et = ps_et.tile([JBLK, 2, CHUNK], F32)
                    for t in range(2):
                        jb = 2 * g + t
                        nc.tensor.matmul(
                            out=et[:, t, :],
                            lhsT=xf_sb[:, jb * JBLK : (jb + 1) * JBLK],
                            rhs=qk_sb,
                            start=True, stop=True,
                        )
                    p_t = ppool.tile([JBLK, 2, CHUNK], F8)
                    if g in DVE_GROUPS:
                        # byte = clamp(round(K8*e + 40)) == fp8e4(exp(e)/4)
                        nc.vector.tensor_scalar(
                            out=p_t[:, :, :].bitcast(U8), in0=et[:, :, :],
                            scalar1=K8, scalar2=0.0,
                            op0=mybir.AluOpType.mult, op1=mybir.AluOpType.max,
                        )
                    else:
                        nc.scalar.activation(
                            out=p_t[:, :, :], in_=et[:, :, :],
                            func=mybir.ActivationFunctionType.Exp,
                            bias=actb_sb[:, 0:1], scale=1.0,
                        )
                    if pend is not None:
                        pp, gg = pend
                        nc.tensor.matmul(
                            out=pv[:, :], lhsT=v_sb[:, gg, :, 0:66], rhs=pp[:, :, :],
                            start=(gg == 0), stop=False,
                            perf_mode=mybir.MatmulPerfMode.DoubleRow,
                        )
                    pend = (p_t, g)
                pp, gg = pend
                nc.tensor.matmul(
                    out=pv[:, :], lhsT=v_sb[:, gg, :, 0:66], rhs=pp[:, :, :],
                    start=False, stop=True,
                    perf_mode=mybir.MatmulPerfMode.DoubleRow,
                )

                # epilogue: y = pv * (1/rowsum) + residual
                r_t = rpool.tile([1, CHUNK], F32R)
                with nc.allow_low_precision(reason="f32r copy of softmax recip"):
                    nc.vector.reciprocal(out=r_t, in_=pv[64:65, :])
                lrb = ps_misc.tile([C, CHUNK], F32, tag="misc")
                nc.tensor.matmul(
                    out=lrb, lhsT=ones_sb[:, :], rhs=r_t[:, :],
                    start=True, stop=True,
                )
                lrb_sb = lpool.tile([C, CHUNK], F32)
                nc.scalar.copy(lrb_sb, lrb)
                y_t = ypool.tile([C, CHUNK], F32)
                nc.vector.tensor_tensor(
                    out=y_t, in0=pv[0:C, :], in1=lrb_sb, op=mybir.AluOpType.mult
                )
                nc.vector.tensor_tensor(
                    out=y_t, in0=y_t, in1=xf_sb[0:C, isl], op=mybir.AluOpType.add
                )
                eng = nc.sync if ic % 2 == 0 else nc.gpsimd
                eng.dma_start(out=y[:, isl], in_=y_t)

    if split_waits:
        _split_multi_waits(nc)
    return nc


_CACHE = {}


def kernel(**inputs):
    x = np.ascontiguousarray(np.asarray(inputs["x"], dtype=np.float32))
    x_RGB = np.ascontiguousarray(np.asarray(inputs["x_RGB"], dtype=np.float32))
    Wq = np.asarray(inputs["Wq"], dtype=np.float32)
    bq = np.asarray(inputs["bq"], dtype=np.float32)
    Wk = np.asarray(inputs["Wk"], dtype=np.float32)
    Wv = np.asarray(inputs["Wv"], dtype=np.float32)
    bv = np.asarray(inputs["bv"], dtype=np.float32)
    lam = np.asarray(inputs["lam"], dtype=np.float32)

    M = (Wq.T.astype(np.float64) @ Wk.astype(np.float64)).astype(np.float32)
    bqk = (Wk.T.astype(np.float64) @ bq.astype(np.float64)).astype(np.float32)

    ls = float(lam.reshape(-1)[0]) * VSCALE
    wv_aug = np.zeros((C + 1, 66), np.float32)
    wv_aug[:C, :C] = Wv.T * ls
    wv_aug[C, :C] = bv * ls
    wv_aug[C, 64] = VSCALE

    cblob = np.zeros((C + 1, 132), np.float32)
    cblob[0:C, 0:64] = M
    cblob[0:C, 65] = bqk
    cblob[C, 65] = BIAS_ROW
    cblob[:, 66:132] = wv_aug

    xf3 = x.reshape(B, C, N)
    xr3 = x_RGB.reshape(B, C, N)

    if "nc" not in _CACHE:
        _CACHE["nc"] = build_bass()
    nc = _CACHE["nc"]

    in_maps = []
    for core in range(NCORES):
        b, ih = core >> 1, core & 1
        xf_aug = np.empty((C + 1, N), np.float32)
        # own query half first (static residual slice), other half after
        xf_aug[:C, :NI] = xf3[b][:, ih * NI : (ih + 1) * NI]
        xf_aug[:C, NI:] = xf3[b][:, (1 - ih) * NI : (2 - ih) * NI]
        xf_aug[C] = 1.0
        in_maps.append(
            {
                "xf": xf_aug.astype(ml_dtypes.bfloat16),
                "xq": np.ascontiguousarray(
                    xr3[b][:, ih * NI : (ih + 1) * NI]
                ).astype(ml_dtypes.bfloat16),
                "cb": cblob.astype(ml_dtypes.bfloat16),
                "onesv": np.ones((1, C), np.float32),
            }
        )

    from concourse.bass_utils import run_bass_kernel_spmd

    res = run_bass_kernel_spmd(nc, in_maps, list(range(NCORES)))

    out = np.empty((B, C, N), np.float32)
    for core in range(NCORES):
        b, ih = core >> 1, core & 1
        out[b][:, ih * NI : (ih + 1) * NI] = res.results[core]["y"]
    return out.reshape(B, C, HH, WW)


# revision 14
# speedup vs baseline: 1.0985x; 1.0985x over previous
"""Trainium2 Bass kernel for the MFPA attention module.

Reference computation (per batch b, with N = H*W = 4096 spatial sites):
    q = Wq @ x_RGB + bq            (CQK=16 channels)
    k = Wk @ x    + bk
    v = Wv @ x    + bv             (C=64 channels)
    energy[i,j] = q_i . k_j
    att = softmax(energy, axis=j)
    out[c,i] = sum_j v[c,j] att[i,j]
    y = lam * out + x

Device strategy (8 NeuronCores): data-parallel over batch (4) x query-row
halves (2).  Each core holds x[b] fully (for K/V and the residual) and its
2048-row query slice, and computes a flash-style streaming softmax so the
4096x4096 energy matrix never leaves PSUM/SBUF.

Host-side weight folding (softmax is shift-invariant, so bk drops out):
    energy[i,j] = (M^T xr_i + bqk) . xf_j    with  M = Wq^T Wk, bqk = Wk^T bq

v2 pipeline notes:
  * exp of the energy is split across BOTH the Scalar (true Exp -> fp8e4)
    and Vector engines (exponent-stuffing: byte = clamp(K8*e + 40) is the
    fp8e4 bit pattern of exp(e)/4, computed by one tensor_scalar).  A
    constant 40/K8 is folded into the energy via the ones row of xf_aug
    (extra qk channel), so the DVE op needs only (mult, max).
  * attention weights p and v are fp8e4; the PV matmul runs DoubleRow
    (256-deep contraction), halving tensor-engine streaming time.
  * the 1/rowsum uses the fast custom-DVE reciprocal; lam and a fp8
    range scale S are folded into wv on the host.
  * residual is taken from the bf16 xf tile (xres input dropped).
  * a short burst of dummy matmuls warms the PE HAM clock-gate while the
    input DMAs stream.
"""

import ml_dtypes
import numpy as np

import concourse.bass as bass
import concourse.mybir as mybir
import concourse.tile as tile_mod
from concourse.vector_clock import ScopedClock

B, C, HH, WW = 4, 64, 64, 64
N = HH * WW          # 4096 spatial sites
NI = N // 2          # query rows per core
CHUNK = 512          # query rows processed per main-loop iteration
NCHUNK = NI // CHUNK
JBLK = 128           # key/value block (PSUM partition dim)
NJP = N // (2 * JBLK)  # 16 j-pair groups (256 keys each)
NCORES = 8
NWARM = 10           # HAM warm-up matmuls

F32 = mybir.dt.float32
F32R = mybir.dt.float32r
BF16 = mybir.dt.bfloat16
F8 = mybir.dt.float8e4
U8 = mybir.dt.uint8
I32 = mybir.dt.int32

K8 = 8.0 / float(np.log(2.0))       # 11.5416  (fp8e4 octave = 8 bytes)
BIAS_B = 40.0                       # byte bias; fp8e4 exp bias is 56 -> p = exp(e)/4
BIAS_ROW = BIAS_B / K8              # energy offset carried by the qk ones-channel
ACT_BIAS = -(BIAS_ROW + float(2.0 * np.log(2.0)))   # Exp(e' + ACT_BIAS) = exp(e)/4
VSCALE = 16.0                       # fp8 range scale on v (cancels in p@v / p@1)
# exp on the vector engine for these j-blocks (14 of 32; DVE also carries
# the bit-trick reciprocal + epilogue muls), scalar takes 18
DVE_BLOCKS = frozenset(range(1, 28, 2))
RMAGIC = float(0x7EF31000)          # bits(1/x) ~= RMAGIC - bits(x), ~5% max err
NWARM2 = 16


def _patched_drain_and_barrier(self, tick_clock, wait_clock):
    # The walrus build in this container rejects instructions with more than
    # one sync-wait command ("Too many sync wait commands" on the Tile tail
    # drain).  Split the aggregated drain into one drain per semaphore wait.
    nc = self.nc
    drain_inst = nc.sync.drain()
    wait_clock.add_sem_waits(
        drain_inst.ins, ScopedClock({None: tick_clock.global_clock})
    )
    inst = drain_inst.ins
    si = inst.sync_info
    waits = list(si.on_wait or []) if si else []
    if len(waits) > 1:
        si.on_wait = waits[:1]
        for w in waits[1:]:
            extra = nc.sync.drain()
            extra.ins.sync_info = mybir.SyncInfo(on_wait=[w], on_update=[])
    nc.all_engine_barrier()
    popped = nc._tile_sem_poison_stack.pop()
    assert popped is self._sem_poison
    nc.clear_and_free_semaphores(list(self.sems.allocated().values()))
    nc.all_engine_barrier()


tile_mod.TileContext._drain_and_barrier = _patched_drain_and_barrier


def _split_multi_waits(nc):
    # This walrus build accepts at most one sync-wait command per TPB
    # instruction.  Hoist extra waits onto engine NoOps placed just before
    # the instruction (engine executes in order, so semantics are kept).
    for blk in nc.m.functions[0].blocks:
        insts = list(blk.instructions)
        out = []
        changed = False
        for inst in insts:
            si = inst.sync_info
            if si is not None and si.on_wait and len(si.on_wait) > 1:
                waits = list(si.on_wait)
                si.on_wait = waits[-1:]
                for w in waits[:-1]:
                    nop = mybir.InstNoOp(name=nc.get_next_instruction_name())
                    nop.engine = inst.engine
                    nop.sync_info = mybir.SyncInfo(on_wait=[w], on_update=[])
                    out.append(nop)
                changed = True
            out.append(inst)
        if changed:
            blk.instructions = out


def build_bass(split_waits=True):
    nc = bass.Bass()
    xf = nc.declare_dram_parameter("xf", [C + 1, N], BF16, isOutput=False)
    xq = nc.declare_dram_parameter("xq", [C, NI], BF16, isOutput=False)
    # consts blob: cols 0:64 = M, col 64 = zero, col 65 = bias, cols 66:132 = wv_aug
    cb = nc.declare_dram_parameter("cb", [C + 1, 132], BF16, isOutput=False)
    y = nc.declare_dram_parameter("y", [C, NI], F32, isOutput=True)

    with tile_mod.TileContext(nc) as tc:
        with (
            tc.tile_pool(name="singles", bufs=1) as singles,
            tc.tile_pool(name="ppool", bufs=4) as ppool,
            tc.tile_pool(name="ypool", bufs=3) as ypool,
            tc.tile_pool(name="lpool", bufs=2) as lpool,
            tc.tile_pool(name="rpool", bufs=2) as rpool,
            tc.tile_pool(name="ps_et", bufs=4, space="PSUM") as ps_et,
            tc.tile_pool(name="ps_pv", bufs=2, space="PSUM") as ps_pv,
            tc.tile_pool(name="ps_misc", bufs=2, space="PSUM") as ps_misc,
        ):
            # ---- tiny SBUF constants (no DMA) -----------------------------
            wsb = singles.tile([128, CHUNK], BF16)
            nc.vector.memset(wsb, 0.0)
            ones_sb = singles.tile([1, C], BF16)
            nc.vector.memset(ones_sb, 1.0)
            actb_sb = singles.tile([128, 1], F32)
            nc.vector.memset(actb_sb, ACT_BIAS)

            # ---- input DMAs: one trigger per tensor/half ------------------
            xq_sb = singles.tile([C, NI], BF16)
            nc.sync.dma_start(out=xq_sb, in_=xq[:, :])
            cb_sb = singles.tile([C + 1, 132], BF16)
            nc.gpsimd.dma_start(out=cb_sb, in_=cb[:, :])
            xf_sb = singles.tile([C + 1, N], BF16)
            nc.gpsimd.dma_start(out=xf_sb[:, 0:NI], in_=xf[:, 0:NI])
            nc.sync.dma_start(out=xf_sb[:, NI:N], in_=xf[:, NI:N])
            # xf arrives column-permuted so this core's own query half sits
            # at columns 0:NI (softmax is j-permutation invariant); row C is
            # all-ones: it feeds the V bias row, the qk energy-bias channel
            # and the fp8 rowsum column.

            # ---- HAM warm-up: a gapless accumulating matmul burst latches
            # the PE clock-gate to full speed while the input DMAs stream
            wp = ps_misc.tile([128, CHUNK], F32, tag="misc")
            for k in range(NWARM2):
                nc.tensor.matmul(
                    out=wp, lhsT=wsb[:, 0:128], rhs=wsb,
                    start=(k == 0), stop=(k == NWARM2 - 1),
                )

            bqk_sb = singles.tile([C + 1, 1], F32)
            nc.vector.tensor_copy(bqk_sb, cb_sb[0 : C + 1, 65:66])

            # ---- Q.K preparation for all chunks ---------------------------
            # qs row C is 0 (M blob column 64..65 region row C is zero), and
            # the bias vector carries BIAS_ROW there, so the Identity
            # activation fills the whole (C+1)-row qk tile in one op.
            qk_sbs = []
            for ic in range(NCHUNK):
                isl = slice(ic * CHUNK, (ic + 1) * CHUNK)
                qs = ps_misc.tile([C + 1, CHUNK], F32, tag="misc")
                nc.tensor.matmul(
                    out=qs, lhsT=cb_sb[0:C, 0:65], rhs=xq_sb[:, isl],
                    start=True, stop=True,
                )
                qk_sb = singles.tile([C + 1, CHUNK], BF16, name=f"qk_sb{ic}")
                nc.scalar.activation(
                    out=qk_sb, in_=qs,
                    func=mybir.ActivationFunctionType.Identity,
                    bias=bqk_sb[:, 0:1], scale=1.0,
                )
                qk_sbs.append(qk_sb)

            # ---- V preparation: fp8 v_pair[jp, pair, t, c] ----------------
            # pair-dim byte step must be 16-aligned for DoubleRow ldweights -> pad 66 to 80
            v_sb = singles.tile([JBLK, NJP, 2, 80], F8)
            for m4 in range(N // (4 * JBLK)):
                vp = ps_misc.tile([JBLK, 2, 2, 66], F32, tag="misc")
                for k in range(4):
                    jb = 4 * m4 + k
                    nc.tensor.matmul(
                        out=vp[:, k // 2, k % 2, :],
                        lhsT=xf_sb[:, jb * JBLK : (jb + 1) * JBLK],
                        rhs=cb_sb[:, 66:132],
                        start=True, stop=True,
                    )
                nc.scalar.activation(
                    out=v_sb[:, 2 * m4 : 2 * m4 + 2, :, 0:66], in_=vp,
                    func=mybir.ActivationFunctionType.Copy,
                )

            # ---- main loop over query chunks ------------------------------
            # Per chunk: 32 j-block energy matmuls -> exp (split ACT/DVE)
            # -> DoubleRow PV accumulation, software-pipelined so the PE
            # never waits: PV of pair jp issues ~2 j-blocks after its exp,
            # and the previous chunk's epilogue tail (which trails the slow
            # reciprocal) is emitted a few groups into the next chunk.
            tail_ops = []
            for ic in range(NCHUNK):
                isl = slice(ic * CHUNK, (ic + 1) * CHUNK)
                qk_sb = qk_sbs[ic]
                pv = ps_pv.tile([66, CHUNK], F32)
                p_t = None
                for jb in range(2 * NJP):
                    et = ps_et.tile([JBLK, CHUNK], F32)
                    nc.tensor.matmul(
                        out=et,
                        lhsT=xf_sb[:, jb * JBLK : (jb + 1) * JBLK],
                        rhs=qk_sb,
                        start=True, stop=True,
                    )
                    if jb % 2 == 0:
                        p_t = ppool.tile([JBLK, 2, CHUNK], F8)
                        p_ts = [p_t] if jb == 0 else p_ts + [p_t]
                    if jb in DVE_BLOCKS:
                        # byte = clamp(round(K8*e + 40)) == fp8e4(exp(e)/4)
                        nc.vector.tensor_scalar(
                            out=p_t[:, jb % 2, :].bitcast(U8), in0=et[:, :],
                            scalar1=K8, scalar2=0.0,
                            op0=mybir.AluOpType.mult, op1=mybir.AluOpType.max,
                        )
                    else:
                        nc.scalar.activation(
                            out=p_t[:, jb % 2, :], in_=et[:, :],
                            func=mybir.ActivationFunctionType.Exp,
                            bias=actb_sb[:, 0:1], scale=1.0,
                        )
                    if jb == 6 and tail_ops:
                        for fn in tail_ops:
                            fn()
                        tail_ops = []
                    if jb >= 5 and (jb - 5) % 2 == 0:
                        jp = (jb - 5) // 2
                        nc.tensor.matmul(
                            out=pv[:, :], lhsT=v_sb[:, jp, :, 0:66],
                            rhs=p_ts[jp][:, :, :],
                            start=(jp == 0), stop=False,
                            perf_mode=mybir.MatmulPerfMode.DoubleRow,
                        )
                for jp in (NJP - 2, NJP - 1):
                    nc.tensor.matmul(
                        out=pv[:, :], lhsT=v_sb[:, jp, :, 0:66],
                        rhs=p_ts[jp][:, :, :],
                        start=False, stop=(jp == NJP - 1),
                        perf_mode=mybir.MatmulPerfMode.DoubleRow,
                    )

                # epilogue: y = pv * (1/rowsum) + residual.  The reciprocal
                # is the exponent-flip bit trick (one cheap DVE op); the
                # broadcast matmul + scale/residual ops are deferred into the
                # next chunk so they never stall the PE or the exp engines.
                r0 = rpool.tile([1, CHUNK], I32)
                nc.vector.tensor_scalar(
                    out=r0, in0=pv[64:65, :].bitcast(I32),
                    scalar1=-1.0, scalar2=RMAGIC,
                    op0=mybir.AluOpType.mult, op1=mybir.AluOpType.add,
                )
                r_bf = rpool.tile([1, CHUNK], BF16)
                nc.gpsimd.tensor_copy(r_bf, r0[:, :].bitcast(F32))

                def make_tail(pv=pv, r_bf=r_bf, isl=isl, ic=ic):
                    def emit():
                        lrb = ps_misc.tile([C, CHUNK], F32, tag="misc")
                        nc.tensor.matmul(
                            out=lrb, lhsT=ones_sb[:, :], rhs=r_bf[:, :],
                            start=True, stop=True,
                        )
                        lrb_sb = lpool.tile([C, CHUNK], F32)
                        nc.scalar.copy(lrb_sb, lrb)
                        y_t = ypool.tile([C, CHUNK], F32)
                        nc.vector.tensor_tensor(
                            out=y_t, in0=pv[0:C, :], in1=lrb_sb,
                            op=mybir.AluOpType.mult,
                        )
                        nc.vector.tensor_tensor(
                            out=y_t, in0=y_t, in1=xf_sb[0:C, isl],
                            op=mybir.AluOpType.add,
                        )
                        eng = nc.sync if ic % 2 == 0 else nc.gpsimd
                        eng.dma_start(out=y[:, isl], in_=y_t)
                    return emit

                tail_ops = [make_tail()]
            for fn in tail_ops:
                fn()

    if split_waits:
        _split_multi_waits(nc)
    return nc


_CACHE = {}


def kernel(**inputs):
    x = np.ascontiguousarray(np.asarray(inputs["x"], dtype=np.float32))
    x_RGB = np.ascontiguousarray(np.asarray(inputs["x_RGB"], dtype=np.float32))
    Wq = np.asarray(inputs["Wq"], dtype=np.float32)
    bq = np.asarray(inputs["bq"], dtype=np.float32)
    Wk = np.asarray(inputs["Wk"], dtype=np.float32)
    Wv = np.asarray(inputs["Wv"], dtype=np.float32)
    bv = np.asarray(inputs["bv"], dtype=np.float32)
    lam = np.asarray(inputs["lam"], dtype=np.float32)

    M = (Wq.T.astype(np.float64) @ Wk.astype(np.float64)).astype(np.float32)
    bqk = (Wk.T.astype(np.float64) @ bq.astype(np.float64)).astype(np.float32)

    ls = float(lam.reshape(-1)[0]) * VSCALE
    wv_aug = np.zeros((C + 1, 66), np.float32)
    wv_aug[:C, :C] = Wv.T * ls
    wv_aug[C, :C] = bv * ls
    wv_aug[C, 64] = VSCALE

    cblob = np.zeros((C + 1, 132), np.float32)
    cblob[0:C, 0:64] = M
    cblob[0:C, 65] = bqk
    cblob[C, 65] = BIAS_ROW
    cblob[:, 66:132] = wv_aug

    xf3 = x.reshape(B, C, N)
    xr3 = x_RGB.reshape(B, C, N)

    if "nc" not in _CACHE:
        _CACHE["nc"] = build_bass()
    nc = _CACHE["nc"]

    in_maps = []
    for core in range(NCORES):
        b, ih = core >> 1, core & 1
        xf_aug = np.empty((C + 1, N), np.float32)
        # own query half first (static residual slice), other half after
        xf_aug[:C, :NI] = xf3[b][:, ih * NI : (ih + 1) * NI]
        xf_aug[:C, NI:] = xf3[b][:, (1 - ih) * NI : (2 - ih) * NI]
        xf_aug[C] = 1.0
        in_maps.append(
            {
                "xf": xf_aug.astype(ml_dtypes.bfloat16),
                "xq": np.ascontiguousarray(
                    xr3[b][:, ih * NI : (ih + 1) * NI]
                ).astype(ml_dtypes.bfloat16),
                "cb": cblob.astype(ml_dtypes.bfloat16),
            }
        )

    from concourse.bass_utils import run_bass_kernel_spmd

    res = run_bass_kernel_spmd(nc, in_maps, list(range(NCORES)))

    out = np.empty((B, C, N), np.float32)
    for core in range(NCORES):
        b, ih = core >> 1, core & 1
        out[b][:, ih * NI : (ih + 1) * NI] = res.results[core]["y"]
    return out.reshape(B, C, HH, WW)


# revision 15
# speedup vs baseline: 1.2483x; 1.1364x over previous
"""Trainium2 Bass kernel for the MFPA attention module.

Reference computation (per batch b, with N = H*W = 4096 spatial sites):
    q = Wq @ x_RGB + bq            (CQK=16 channels)
    k = Wk @ x    + bk
    v = Wv @ x    + bv             (C=64 channels)
    energy[i,j] = q_i . k_j
    att = softmax(energy, axis=j)
    out[c,i] = sum_j v[c,j] att[i,j]
    y = lam * out + x

Device strategy (8 NeuronCores): data-parallel over batch (4) x query-row
halves (2).  Each core holds x[b] fully (for K/V and the residual) and its
2048-row query slice, and computes a flash-style streaming softmax so the
4096x4096 energy matrix never leaves PSUM/SBUF.

Host-side weight folding (softmax is shift-invariant, so bk drops out):
    energy[i,j] = (M^T xr_i + bqk) . xf_j    with  M = Wq^T Wk, bqk = Wk^T bq

v2 pipeline notes:
  * exp of the energy is split across BOTH the Scalar (true Exp -> fp8e4)
    and Vector engines (exponent-stuffing: byte = clamp(K8*e + 40) is the
    fp8e4 bit pattern of exp(e)/4, computed by one tensor_scalar).  A
    constant 40/K8 is folded into the energy via the ones row of xf_aug
    (extra qk channel), so the DVE op needs only (mult, max).
  * attention weights p and v are fp8e4; the PV matmul runs DoubleRow
    (256-deep contraction), halving tensor-engine streaming time.
  * the 1/rowsum uses the fast custom-DVE reciprocal; lam and a fp8
    range scale S are folded into wv on the host.
  * residual is taken from the bf16 xf tile (xres input dropped).
  * a short burst of dummy matmuls warms the PE HAM clock-gate while the
    input DMAs stream.
"""

import ml_dtypes
import numpy as np

import concourse.bass as bass
import concourse.mybir as mybir
import concourse.tile as tile_mod
from concourse.vector_clock import ScopedClock

B, C, HH, WW = 4, 64, 64, 64
N = HH * WW          # 4096 spatial sites
NI = N // 2          # query rows per core
CHUNK = 512          # query rows processed per main-loop iteration
NCHUNK = NI // CHUNK
JBLK = 128           # key/value block (PSUM partition dim)
NJP = N // (2 * JBLK)  # 16 j-pair groups (256 keys each)
NCORES = 8
NWARM = 10           # HAM warm-up matmuls

F32 = mybir.dt.float32
F32R = mybir.dt.float32r
BF16 = mybir.dt.bfloat16
F8 = mybir.dt.float8e4
U8 = mybir.dt.uint8
I32 = mybir.dt.int32

K8 = 8.0 / float(np.log(2.0))       # 11.5416  (fp8e4 octave = 8 bytes)
BIAS_B = 40.0                       # byte bias; fp8e4 exp bias is 56 -> p = exp(e)/4
BIAS_ROW = BIAS_B / K8              # energy offset carried by the qk ones-channel
ACT_BIAS = -(BIAS_ROW + float(2.0 * np.log(2.0)))   # Exp(e' + ACT_BIAS) = exp(e)/4
VSCALE = 16.0                       # fp8 range scale on v (cancels in p@v / p@1)
# exp on the vector engine for these j-blocks (14 of 32; DVE also carries
# the bit-trick reciprocal + epilogue muls), scalar takes 18
DVE_BLOCKS = frozenset(range(1, 28, 2))
RMAGIC = float(0x7EF31000)          # bits(1/x) ~= RMAGIC - bits(x), ~5% max err
NWARM2 = 16


def _patched_drain_and_barrier(self, tick_clock, wait_clock):
    # The walrus build in this container rejects instructions with more than
    # one sync-wait command ("Too many sync wait commands" on the Tile tail
    # drain).  Split the aggregated drain into one drain per semaphore wait.
    nc = self.nc
    drain_inst = nc.sync.drain()
    wait_clock.add_sem_waits(
        drain_inst.ins, ScopedClock({None: tick_clock.global_clock})
    )
    inst = drain_inst.ins
    si = inst.sync_info
    waits = list(si.on_wait or []) if si else []
    if len(waits) > 1:
        si.on_wait = waits[:1]
        for w in waits[1:]:
            extra = nc.sync.drain()
            extra.ins.sync_info = mybir.SyncInfo(on_wait=[w], on_update=[])
    nc.all_engine_barrier()
    popped = nc._tile_sem_poison_stack.pop()
    assert popped is self._sem_poison
    nc.clear_and_free_semaphores(list(self.sems.allocated().values()))
    nc.all_engine_barrier()


tile_mod.TileContext._drain_and_barrier = _patched_drain_and_barrier


def _split_multi_waits(nc):
    # This walrus build accepts at most one sync-wait command per TPB
    # instruction.  Hoist extra waits onto engine NoOps placed just before
    # the instruction (engine executes in order, so semantics are kept).
    for blk in nc.m.functions[0].blocks:
        insts = list(blk.instructions)
        out = []
        changed = False
        for inst in insts:
            si = inst.sync_info
            if si is not None and si.on_wait and len(si.on_wait) > 1:
                waits = list(si.on_wait)
                si.on_wait = waits[-1:]
                for w in waits[:-1]:
                    nop = mybir.InstNoOp(name=nc.get_next_instruction_name())
                    nop.engine = inst.engine
                    nop.sync_info = mybir.SyncInfo(on_wait=[w], on_update=[])
                    out.append(nop)
                changed = True
            out.append(inst)
        if changed:
            blk.instructions = out


def build_bass(split_waits=True):
    nc = bass.Bass()
    xf = nc.declare_dram_parameter("xf", [C + 1, N], BF16, isOutput=False)
    xq = nc.declare_dram_parameter("xq", [C, NI], BF16, isOutput=False)
    # consts blob: cols 0:64 = M, col 64 = zero, col 65 = bias, cols 66:132 = wv_aug
    cb = nc.declare_dram_parameter("cb", [C + 1, 132], BF16, isOutput=False)
    y = nc.declare_dram_parameter("y", [C, NI], F32, isOutput=True)

    with tile_mod.TileContext(nc) as tc:
        with (
            tc.tile_pool(name="singles", bufs=1) as singles,
            tc.tile_pool(name="ppool", bufs=4) as ppool,
            tc.tile_pool(name="ypool", bufs=3) as ypool,
            tc.tile_pool(name="lpool", bufs=2) as lpool,
            tc.tile_pool(name="rpool", bufs=2) as rpool,
            tc.tile_pool(name="ps_et", bufs=4, space="PSUM") as ps_et,
            tc.tile_pool(name="ps_pv", bufs=2, space="PSUM") as ps_pv,
            tc.tile_pool(name="ps_misc", bufs=2, space="PSUM") as ps_misc,
        ):
            # ---- tiny SBUF constants (no DMA) -----------------------------
            wsb = singles.tile([128, CHUNK], BF16)
            nc.vector.memset(wsb, 0.0)
            ones_sb = singles.tile([1, C], BF16)
            nc.vector.memset(ones_sb, 1.0)
            actb_sb = singles.tile([128, 1], F32)
            nc.vector.memset(actb_sb, ACT_BIAS)
            # the first ACTIVATE pays the ~2.7us exp table load; fire it on a
            # dummy tile immediately so it overlaps the input DMAs/warm-up
            tblw = singles.tile([128, 1], F32)
            nc.scalar.activation(
                out=tblw, in_=actb_sb,
                func=mybir.ActivationFunctionType.Exp,
            )

            # ---- input DMAs: one trigger per tensor/half ------------------
            xq_sb = singles.tile([C, NI], BF16)
            nc.sync.dma_start(out=xq_sb, in_=xq[:, :])
            cb_sb = singles.tile([C + 1, 132], BF16)
            nc.gpsimd.dma_start(out=cb_sb, in_=cb[:, :])
            xf_sb = singles.tile([C + 1, N], BF16)
            nc.gpsimd.dma_start(out=xf_sb[:, 0:NI], in_=xf[:, 0:NI])
            nc.sync.dma_start(out=xf_sb[:, NI:N], in_=xf[:, NI:N])
            # xf arrives column-permuted so this core's own query half sits
            # at columns 0:NI (softmax is j-permutation invariant); row C is
            # all-ones: it feeds the V bias row, the qk energy-bias channel
            # and the fp8 rowsum column.

            # ---- HAM warm-up: a gapless accumulating matmul burst latches
            # the PE clock-gate to full speed while the input DMAs stream
            wp = ps_misc.tile([128, CHUNK], F32, tag="misc")
            for k in range(NWARM2):
                nc.tensor.matmul(
                    out=wp, lhsT=wsb[:, 0:128], rhs=wsb,
                    start=(k == 0), stop=(k == NWARM2 - 1),
                )

            bqk_sb = singles.tile([C + 1, 1], F32)
            nc.vector.tensor_copy(bqk_sb, cb_sb[0 : C + 1, 65:66])

            # ---- Q.K preparation -----------------------------------------
            # qs row C is 0 (M blob column 64..65 region row C is zero), and
            # the bias vector carries BIAS_ROW there, so the Identity
            # activation fills the whole (C+1)-row qk tile in one op.
            # Only chunk 0 is prepared up front; chunk ic+1's qk is produced
            # inside chunk ic's block stream so the PE never idles on it.
            qk_sbs = [
                singles.tile([C + 1, CHUNK], BF16, name=f"qk_sb{ic}")
                for ic in range(NCHUNK)
            ]

            def emit_qk_prep(ic):
                isl = slice(ic * CHUNK, (ic + 1) * CHUNK)
                qs = ps_misc.tile([C + 1, CHUNK], F32, tag="misc")
                nc.tensor.matmul(
                    out=qs, lhsT=cb_sb[0:C, 0:65], rhs=xq_sb[:, isl],
                    start=True, stop=True,
                )
                nc.scalar.activation(
                    out=qk_sbs[ic], in_=qs,
                    func=mybir.ActivationFunctionType.Identity,
                    bias=bqk_sb[:, 0:1], scale=1.0,
                )

            emit_qk_prep(0)

            # ---- V preparation: fp8 v_pair[jp, pair, t, c] ----------------
            # pair-dim byte step must be 16-aligned for DoubleRow ldweights -> pad 66 to 80
            v_sb = singles.tile([JBLK, NJP, 2, 80], F8)
            for m4 in range(N // (4 * JBLK)):
                vp = ps_misc.tile([JBLK, 2, 2, 66], F32, tag="misc")
                for k in range(4):
                    jb = 4 * m4 + k
                    nc.tensor.matmul(
                        out=vp[:, k // 2, k % 2, :],
                        lhsT=xf_sb[:, jb * JBLK : (jb + 1) * JBLK],
                        rhs=cb_sb[:, 66:132],
                        start=True, stop=True,
                    )
                nc.scalar.activation(
                    out=v_sb[:, 2 * m4 : 2 * m4 + 2, :, 0:66], in_=vp,
                    func=mybir.ActivationFunctionType.Copy,
                )

            # ---- main loop over query chunks ------------------------------
            # Per chunk: 32 j-block energy matmuls -> exp (split ACT/DVE)
            # -> DoubleRow PV accumulation, software-pipelined so the PE
            # never waits: PV of pair jp issues ~2 j-blocks after its exp,
            # and the previous chunk's epilogue tail (which trails the slow
            # reciprocal) is emitted a few groups into the next chunk.
            tail_ops = []
            for ic in range(NCHUNK):
                isl = slice(ic * CHUNK, (ic + 1) * CHUNK)
                qk_sb = qk_sbs[ic]
                pv = ps_pv.tile([66, CHUNK], F32)
                p_t = None
                for jb in range(2 * NJP):
                    et = ps_et.tile([JBLK, CHUNK], F32)
                    nc.tensor.matmul(
                        out=et,
                        lhsT=xf_sb[:, jb * JBLK : (jb + 1) * JBLK],
                        rhs=qk_sb,
                        start=True, stop=True,
                    )
                    if jb % 2 == 0:
                        p_t = ppool.tile([JBLK, 2, CHUNK], F8)
                        p_ts = [p_t] if jb == 0 else p_ts + [p_t]
                    if jb in DVE_BLOCKS:
                        # byte = clamp(round(K8*e + 40)) == fp8e4(exp(e)/4)
                        nc.vector.tensor_scalar(
                            out=p_t[:, jb % 2, :].bitcast(U8), in0=et[:, :],
                            scalar1=K8, scalar2=0.0,
                            op0=mybir.AluOpType.mult, op1=mybir.AluOpType.max,
                        )
                    else:
                        nc.scalar.activation(
                            out=p_t[:, jb % 2, :], in_=et[:, :],
                            func=mybir.ActivationFunctionType.Exp,
                            bias=actb_sb[:, 0:1], scale=1.0,
                        )
                    if jb == 6 and tail_ops:
                        for fn in tail_ops:
                            fn()
                        tail_ops = []
                    if jb == 12 and ic + 1 < NCHUNK:
                        emit_qk_prep(ic + 1)
                    if jb >= 5 and (jb - 5) % 2 == 0:
                        jp = (jb - 5) // 2
                        nc.tensor.matmul(
                            out=pv[:, :], lhsT=v_sb[:, jp, :, 0:66],
                            rhs=p_ts[jp][:, :, :],
                            start=(jp == 0), stop=False,
                            perf_mode=mybir.MatmulPerfMode.DoubleRow,
                        )
                for jp in (NJP - 2, NJP - 1):
                    nc.tensor.matmul(
                        out=pv[:, :], lhsT=v_sb[:, jp, :, 0:66],
                        rhs=p_ts[jp][:, :, :],
                        start=False, stop=(jp == NJP - 1),
                        perf_mode=mybir.MatmulPerfMode.DoubleRow,
                    )

                # epilogue: y = pv * (1/rowsum) + residual.  The reciprocal
                # is the exponent-flip bit trick (one cheap DVE op); the
                # broadcast matmul + scale/residual ops are deferred into the
                # next chunk so they never stall the PE or the exp engines.
                r0 = rpool.tile([1, CHUNK], I32)
                nc.vector.tensor_scalar(
                    out=r0, in0=pv[64:65, :].bitcast(I32),
                    scalar1=-1.0, scalar2=RMAGIC,
                    op0=mybir.AluOpType.mult, op1=mybir.AluOpType.add,
                )
                r_bf = rpool.tile([1, CHUNK], BF16)
                nc.gpsimd.tensor_copy(r_bf, r0[:, :].bitcast(F32))

                def make_tail(pv=pv, r_bf=r_bf, isl=isl, ic=ic):
                    def emit():
                        lrb = ps_misc.tile([C, CHUNK], F32, tag="misc")
                        nc.tensor.matmul(
                            out=lrb, lhsT=ones_sb[:, :], rhs=r_bf[:, :],
                            start=True, stop=True,
                        )
                        lrb_sb = lpool.tile([C, CHUNK], F32)
                        nc.scalar.copy(lrb_sb, lrb)
                        y_t = ypool.tile([C, CHUNK], F32)
                        nc.vector.tensor_tensor(
                            out=y_t, in0=pv[0:C, :], in1=lrb_sb,
                            op=mybir.AluOpType.mult,
                        )
                        nc.vector.tensor_tensor(
                            out=y_t, in0=y_t, in1=xf_sb[0:C, isl],
                            op=mybir.AluOpType.add,
                        )
                        eng = nc.sync if ic % 2 == 0 else nc.gpsimd
                        eng.dma_start(out=y[:, isl], in_=y_t)
                    return emit

                tail_ops = [make_tail()]
            for fn in tail_ops:
                fn()

    if split_waits:
        _split_multi_waits(nc)
    return nc


_CACHE = {}


def kernel(**inputs):
    x = np.ascontiguousarray(np.asarray(inputs["x"], dtype=np.float32))
    x_RGB = np.ascontiguousarray(np.asarray(inputs["x_RGB"], dtype=np.float32))
    Wq = np.asarray(inputs["Wq"], dtype=np.float32)
    bq = np.asarray(inputs["bq"], dtype=np.float32)
    Wk = np.asarray(inputs["Wk"], dtype=np.float32)
    Wv = np.asarray(inputs["Wv"], dtype=np.float32)
    bv = np.asarray(inputs["bv"], dtype=np.float32)
    lam = np.asarray(inputs["lam"], dtype=np.float32)

    M = (Wq.T.astype(np.float64) @ Wk.astype(np.float64)).astype(np.float32)
    bqk = (Wk.T.astype(np.float64) @ bq.astype(np.float64)).astype(np.float32)

    ls = float(lam.reshape(-1)[0]) * VSCALE
    wv_aug = np.zeros((C + 1, 66), np.float32)
    wv_aug[:C, :C] = Wv.T * ls
    wv_aug[C, :C] = bv * ls
    wv_aug[C, 64] = VSCALE

    cblob = np.zeros((C + 1, 132), np.float32)
    cblob[0:C, 0:64] = M
    cblob[0:C, 65] = bqk
    cblob[C, 65] = BIAS_ROW
    cblob[:, 66:132] = wv_aug

    xf3 = x.reshape(B, C, N)
    xr3 = x_RGB.reshape(B, C, N)

    if "nc" not in _CACHE:
        _CACHE["nc"] = build_bass()
    nc = _CACHE["nc"]

    in_maps = []
    for core in range(NCORES):
        b, ih = core >> 1, core & 1
        xf_aug = np.empty((C + 1, N), np.float32)
        # own query half first (static residual slice), other half after
        xf_aug[:C, :NI] = xf3[b][:, ih * NI : (ih + 1) * NI]
        xf_aug[:C, NI:] = xf3[b][:, (1 - ih) * NI : (2 - ih) * NI]
        xf_aug[C] = 1.0
        in_maps.append(
            {
                "xf": xf_aug.astype(ml_dtypes.bfloat16),
                "xq": np.ascontiguousarray(
                    xr3[b][:, ih * NI : (ih + 1) * NI]
                ).astype(ml_dtypes.bfloat16),
                "cb": cblob.astype(ml_dtypes.bfloat16),
            }
        )

    from concourse.bass_utils import run_bass_kernel_spmd

    res = run_bass_kernel_spmd(nc, in_maps, list(range(NCORES)))

    out = np.empty((B, C, N), np.float32)
    for core in range(NCORES):
        b, ih = core >> 1, core & 1
        out[b][:, ih * NI : (ih + 1) * NI] = res.results[core]["y"]
    return out.reshape(B, C, HH, WW)


# revision 18
# speedup vs baseline: 1.4626x; 1.1717x over previous
"""Trainium2 Bass kernel for the MFPA attention module.

Reference computation (per batch b, with N = H*W = 4096 spatial sites):
    q = Wq @ x_RGB + bq            (CQK=16 channels)
    k = Wk @ x    + bk
    v = Wv @ x    + bv             (C=64 channels)
    energy[i,j] = q_i . k_j
    att = softmax(energy, axis=j)
    out[c,i] = sum_j v[c,j] att[i,j]
    y = lam * out + x

Device strategy (8 NeuronCores): data-parallel over batch (4) x query-row
halves (2).  Each core holds x[b] fully (for K/V and the residual) and its
2048-row query slice, and computes a flash-style streaming softmax so the
4096x4096 energy matrix never leaves PSUM/SBUF.

Host-side weight folding (softmax is shift-invariant, so bk drops out):
    energy[i,j] = (M^T xr_i + bqk) . xf_j    with  M = Wq^T Wk, bqk = Wk^T bq

v2 pipeline notes:
  * exp of the energy is split across BOTH the Scalar (true Exp -> fp8e4)
    and Vector engines (exponent-stuffing: byte = clamp(K8*e + 40) is the
    fp8e4 bit pattern of exp(e)/4, computed by one tensor_scalar).  A
    constant 40/K8 is folded into the energy via the ones row of xf_aug
    (extra qk channel), so the DVE op needs only (mult, max).
  * attention weights p and v are fp8e4; the PV matmul runs DoubleRow
    (256-deep contraction), halving tensor-engine streaming time.
  * the 1/rowsum uses the fast custom-DVE reciprocal; lam and a fp8
    range scale S are folded into wv on the host.
  * residual is taken from the bf16 xf tile (xres input dropped).
  * a short burst of dummy matmuls warms the PE HAM clock-gate while the
    input DMAs stream.
"""

import ml_dtypes
import numpy as np

import concourse.bass as bass
import concourse.mybir as mybir
import concourse.tile as tile_mod
from concourse.vector_clock import ScopedClock

B, C, HH, WW = 4, 64, 64, 64
N = HH * WW          # 4096 spatial sites
NI = N // 2          # query rows per core
CHUNK = 512          # query rows processed per main-loop iteration
NCHUNK = NI // CHUNK
JBLK = 128           # key/value block (PSUM partition dim)
NJP = N // (2 * JBLK)  # 16 j-pair groups (256 keys each)
NCORES = 8
NWARM = 10           # HAM warm-up matmuls

F32 = mybir.dt.float32
F32R = mybir.dt.float32r
BF16 = mybir.dt.bfloat16
F8 = mybir.dt.float8e4
U8 = mybir.dt.uint8
I32 = mybir.dt.int32

K8 = 8.0 / float(np.log(2.0))       # 11.5416  (fp8e4 octave = 8 bytes)
BIAS_B = 40.0                       # byte bias; fp8e4 exp bias is 56 -> p = exp(e)/4
BIAS_ROW = BIAS_B / K8              # energy offset carried by the qk ones-channel
ACT_BIAS = -(BIAS_ROW + float(2.0 * np.log(2.0)))   # Exp(e' + ACT_BIAS) = exp(e)/4
VSCALE = 16.0                       # fp8 range scale on v (cancels in p@v / p@1)
# exp on the vector engine for these j-blocks (14 of 32; DVE also carries
# the bit-trick reciprocal + epilogue muls), scalar takes 18
DVE_BLOCKS = frozenset((1, 3, 5, 7, 9, 11, 13, 19, 21, 23, 25, 27, 29, 31))
RMAGIC = float(0x7EF31000)          # bits(1/x) ~= RMAGIC - bits(x), ~5% max err
NWARM2 = 16


def _patched_drain_and_barrier(self, tick_clock, wait_clock):
    # The walrus build in this container rejects instructions with more than
    # one sync-wait command ("Too many sync wait commands" on the Tile tail
    # drain).  Split the aggregated drain into one drain per semaphore wait.
    nc = self.nc
    drain_inst = nc.sync.drain()
    wait_clock.add_sem_waits(
        drain_inst.ins, ScopedClock({None: tick_clock.global_clock})
    )
    inst = drain_inst.ins
    si = inst.sync_info
    waits = list(si.on_wait or []) if si else []
    if len(waits) > 1:
        si.on_wait = waits[:1]
        for w in waits[1:]:
            extra = nc.sync.drain()
            extra.ins.sync_info = mybir.SyncInfo(on_wait=[w], on_update=[])
    nc.all_engine_barrier()
    popped = nc._tile_sem_poison_stack.pop()
    assert popped is self._sem_poison
    nc.clear_and_free_semaphores(list(self.sems.allocated().values()))
    nc.all_engine_barrier()


tile_mod.TileContext._drain_and_barrier = _patched_drain_and_barrier


def _split_multi_waits(nc):
    # This walrus build accepts at most one sync-wait command per TPB
    # instruction.  Hoist extra waits onto engine NoOps placed just before
    # the instruction (engine executes in order, so semantics are kept).
    for blk in nc.m.functions[0].blocks:
        insts = list(blk.instructions)
        out = []
        changed = False
        for inst in insts:
            si = inst.sync_info
            if si is not None and si.on_wait and len(si.on_wait) > 1:
                waits = list(si.on_wait)
                si.on_wait = waits[-1:]
                for w in waits[:-1]:
                    nop = mybir.InstNoOp(name=nc.get_next_instruction_name())
                    nop.engine = inst.engine
                    nop.sync_info = mybir.SyncInfo(on_wait=[w], on_update=[])
                    out.append(nop)
                changed = True
            out.append(inst)
        if changed:
            blk.instructions = out


def build_bass(split_waits=True):
    nc = bass.Bass()
    xf = nc.declare_dram_parameter("xf", [C + 1, N], BF16, isOutput=False)
    xq = nc.declare_dram_parameter("xq", [C, NI], BF16, isOutput=False)
    # consts blob: cols 0:64 = M, col 64 = zero, col 65 = bias, cols 66:132 = wv_aug
    cb = nc.declare_dram_parameter("cb", [128, 132], BF16, isOutput=False)
    y = nc.declare_dram_parameter("y", [C, NI], F32, isOutput=True)

    with tile_mod.TileContext(nc) as tc:
        with (
            tc.tile_pool(name="singles", bufs=1) as singles,
            tc.tile_pool(name="ppool", bufs=4) as ppool,
            tc.tile_pool(name="ypool", bufs=3) as ypool,
            tc.tile_pool(name="lpool", bufs=2) as lpool,
            tc.tile_pool(name="rpool", bufs=2) as rpool,
            tc.tile_pool(name="ps_et", bufs=4, space="PSUM") as ps_et,
            tc.tile_pool(name="ps_pv", bufs=2, space="PSUM") as ps_pv,
            tc.tile_pool(name="ps_misc", bufs=2, space="PSUM") as ps_misc,
        ):
            # ---- tiny SBUF constants (no DMA) -----------------------------
            wsb = singles.tile([128, CHUNK], BF16)
            nc.vector.memset(wsb, 0.0)
            ones_sb = singles.tile([1, C], BF16)
            nc.vector.memset(ones_sb, 1.0)
            actb_sb = singles.tile([128, 1], F32)
            nc.vector.memset(actb_sb, ACT_BIAS)
            # the first ACTIVATE pays the ~2.7us exp table load; fire it on a
            # dummy tile immediately so it overlaps the input DMAs/warm-up
            tblw = singles.tile([128, 1], F32)
            nc.scalar.activation(
                out=tblw, in_=actb_sb,
                func=mybir.ActivationFunctionType.Exp,
            )

            # ---- input DMAs: one trigger per tensor/half ------------------
            xq_sb = singles.tile([C, NI], BF16)
            nc.sync.dma_start(out=xq_sb, in_=xq[:, :])
            cb_sb = singles.tile([128, 132], BF16)
            nc.gpsimd.dma_start(out=cb_sb, in_=cb[:, :])
            # K=128 contraction everywhere (the fast weight-load path needs
            # full-height weights); rows C+1..127 are zero on the qk side.
            xf_sb = singles.tile([128, N], BF16)
            nc.vector.memset(xf_sb[64:128, :], 0.0)
            nc.gpsimd.dma_start(out=xf_sb[0 : C + 1, 0:NI], in_=xf[:, 0:NI])
            nc.sync.dma_start(out=xf_sb[0 : C + 1, NI:N], in_=xf[:, NI:N])
            # xf arrives column-permuted so this core's own query half sits
            # at columns 0:NI (softmax is j-permutation invariant); row C is
            # all-ones: it feeds the V bias row, the qk energy-bias channel
            # and the fp8 rowsum column.

            # ---- HAM warm-up: a gapless accumulating matmul burst latches
            # the PE clock-gate to full speed while the input DMAs stream
            wp = ps_misc.tile([128, CHUNK], F32, tag="misc")
            for k in range(NWARM2):
                nc.tensor.matmul(
                    out=wp, lhsT=wsb[:, 0:128], rhs=wsb,
                    start=(k == 0), stop=(k == NWARM2 - 1),
                )

            bqk_sb = singles.tile([C + 1, 1], F32)
            nc.vector.tensor_copy(bqk_sb, cb_sb[0 : C + 1, 65:66])

            # ---- Q.K preparation -----------------------------------------
            # qs row C is 0 (M blob column 64..65 region row C is zero), and
            # the bias vector carries BIAS_ROW there, so the Identity
            # activation fills the whole (C+1)-row qk tile in one op.
            # Only chunk 0 is prepared up front; chunk ic+1's qk is produced
            # inside chunk ic's block stream so the PE never idles on it.
            qk_sbs = [
                singles.tile([128, CHUNK], BF16, name=f"qk_sb{ic}")
                for ic in range(NCHUNK)
            ]
            for t in qk_sbs:
                nc.vector.memset(t[64:128, :], 0.0)

            def emit_qk_prep(ic):
                isl = slice(ic * CHUNK, (ic + 1) * CHUNK)
                qs = ps_misc.tile([C + 1, CHUNK], F32, tag="misc")
                nc.tensor.matmul(
                    out=qs, lhsT=cb_sb[0:C, 0:65], rhs=xq_sb[:, isl],
                    start=True, stop=True,
                )
                nc.scalar.activation(
                    out=qk_sbs[ic][0 : C + 1, :], in_=qs,
                    func=mybir.ActivationFunctionType.Identity,
                    bias=bqk_sb[:, 0:1], scale=1.0,
                )

            emit_qk_prep(0)

            # ---- V preparation: fp8 v_pair[jp, pair, t, c] ----------------
            # pair-dim byte step must be 16-aligned for DoubleRow ldweights -> pad 66 to 80
            v_sb = singles.tile([JBLK, NJP, 2, 80], F8)
            for m4 in range(N // (4 * JBLK)):
                vp = ps_misc.tile([JBLK, 2, 2, 66], F32, tag="misc")
                for k in range(4):
                    jb = 4 * m4 + k
                    nc.tensor.matmul(
                        out=vp[:, k // 2, k % 2, :],
                        lhsT=xf_sb[:, jb * JBLK : (jb + 1) * JBLK],
                        rhs=cb_sb[0:128, 66:132],
                        start=True, stop=True,
                    )
                nc.scalar.activation(
                    out=v_sb[:, 2 * m4 : 2 * m4 + 2, :, 0:66], in_=vp,
                    func=mybir.ActivationFunctionType.Copy,
                )

            # ---- main loop over query chunks ------------------------------
            # Per chunk: 32 j-block energy matmuls -> exp (split ACT/DVE)
            # -> DoubleRow PV accumulation, software-pipelined so the PE
            # never waits: PV of pair jp issues ~2 j-blocks after its exp,
            # and the previous chunk's epilogue tail (which trails the slow
            # reciprocal) is emitted a few groups into the next chunk.
            tail_ops = []
            for ic in range(NCHUNK):
                isl = slice(ic * CHUNK, (ic + 1) * CHUNK)
                qk_sb = qk_sbs[ic]
                pv = ps_pv.tile([66, CHUNK], F32)
                p_t = None
                for jb in range(2 * NJP):
                    et = ps_et.tile([JBLK, CHUNK], F32)
                    nc.tensor.matmul(
                        out=et,
                        lhsT=xf_sb[:, jb * JBLK : (jb + 1) * JBLK],
                        rhs=qk_sb,
                        start=True, stop=True,
                    )
                    if jb % 2 == 0:
                        p_t = ppool.tile([JBLK, 2, CHUNK], F8)
                        p_ts = [p_t] if jb == 0 else p_ts + [p_t]
                    if jb in DVE_BLOCKS:
                        # byte = clamp(round(K8*e + 40)) == fp8e4(exp(e)/4)
                        nc.vector.tensor_scalar(
                            out=p_t[:, jb % 2, :].bitcast(U8), in0=et[:, :],
                            scalar1=K8, scalar2=0.0,
                            op0=mybir.AluOpType.mult, op1=mybir.AluOpType.max,
                        )
                    else:
                        nc.scalar.activation(
                            out=p_t[:, jb % 2, :], in_=et[:, :],
                            func=mybir.ActivationFunctionType.Exp,
                            bias=actb_sb[:, 0:1], scale=1.0,
                        )
                    if jb == 6 and tail_ops:
                        for fn in tail_ops:
                            fn()
                        tail_ops = []
                    if jb == 12 and ic + 1 < NCHUNK:
                        emit_qk_prep(ic + 1)
                    if jb >= 5 and (jb - 5) % 2 == 0:
                        jp = (jb - 5) // 2
                        nc.tensor.matmul(
                            out=pv[:, :], lhsT=v_sb[:, jp, :, 0:66],
                            rhs=p_ts[jp][:, :, :],
                            start=(jp == 0), stop=False,
                            perf_mode=mybir.MatmulPerfMode.DoubleRow,
                        )
                for jp in (NJP - 2, NJP - 1):
                    nc.tensor.matmul(
                        out=pv[:, :], lhsT=v_sb[:, jp, :, 0:66],
                        rhs=p_ts[jp][:, :, :],
                        start=False, stop=(jp == NJP - 1),
                        perf_mode=mybir.MatmulPerfMode.DoubleRow,
                    )

                # epilogue: y = pv * (1/rowsum) + residual.  The reciprocal
                # is the exponent-flip bit trick (one cheap DVE op); the
                # broadcast matmul + scale/residual ops are deferred into the
                # next chunk so they never stall the PE or the exp engines.
                r0 = rpool.tile([1, CHUNK], I32)
                nc.vector.tensor_scalar(
                    out=r0, in0=pv[64:65, :].bitcast(I32),
                    scalar1=-1.0, scalar2=RMAGIC,
                    op0=mybir.AluOpType.mult, op1=mybir.AluOpType.add,
                )
                r_bf = rpool.tile([1, CHUNK], BF16)
                nc.gpsimd.tensor_copy(r_bf, r0[:, :].bitcast(F32))

                def make_tail(pv=pv, r_bf=r_bf, isl=isl, ic=ic):
                    def emit():
                        lrb = ps_misc.tile([C, CHUNK], F32, tag="misc")
                        nc.tensor.matmul(
                            out=lrb, lhsT=ones_sb[:, :], rhs=r_bf[:, :],
                            start=True, stop=True,
                        )
                        lrb_sb = lpool.tile([C, CHUNK], F32)
                        nc.scalar.copy(lrb_sb, lrb)
                        y_t = ypool.tile([C, CHUNK], F32)
                        nc.vector.tensor_tensor(
                            out=y_t, in0=pv[0:C, :], in1=lrb_sb,
                            op=mybir.AluOpType.mult,
                        )
                        nc.vector.tensor_tensor(
                            out=y_t, in0=y_t, in1=xf_sb[0:C, isl],
                            op=mybir.AluOpType.add,
                        )
                        eng = nc.sync if ic % 2 == 0 else nc.gpsimd
                        eng.dma_start(out=y[:, isl], in_=y_t)
                    return emit

                tail_ops = [make_tail()]
            for fn in tail_ops:
                fn()

    if split_waits:
        _split_multi_waits(nc)
    return nc


_CACHE = {}


def kernel(**inputs):
    x = np.ascontiguousarray(np.asarray(inputs["x"], dtype=np.float32))
    x_RGB = np.ascontiguousarray(np.asarray(inputs["x_RGB"], dtype=np.float32))
    Wq = np.asarray(inputs["Wq"], dtype=np.float32)
    bq = np.asarray(inputs["bq"], dtype=np.float32)
    Wk = np.asarray(inputs["Wk"], dtype=np.float32)
    Wv = np.asarray(inputs["Wv"], dtype=np.float32)
    bv = np.asarray(inputs["bv"], dtype=np.float32)
    lam = np.asarray(inputs["lam"], dtype=np.float32)

    M = (Wq.T.astype(np.float64) @ Wk.astype(np.float64)).astype(np.float32)
    bqk = (Wk.T.astype(np.float64) @ bq.astype(np.float64)).astype(np.float32)

    ls = float(lam.reshape(-1)[0]) * VSCALE
    wv_aug = np.zeros((C + 1, 66), np.float32)
    wv_aug[:C, :C] = Wv.T * ls
    wv_aug[C, :C] = bv * ls
    wv_aug[C, 64] = VSCALE

    cblob = np.zeros((128, 132), np.float32)
    cblob[0:C, 0:64] = M
    cblob[0:C, 65] = bqk
    cblob[C, 65] = BIAS_ROW
    cblob[0 : C + 1, 66:132] = wv_aug

    xf3 = x.reshape(B, C, N)
    xr3 = x_RGB.reshape(B, C, N)

    if "nc" not in _CACHE:
        _CACHE["nc"] = build_bass()
    nc = _CACHE["nc"]

    in_maps = []
    for core in range(NCORES):
        b, ih = core >> 1, core & 1
        xf_aug = np.empty((C + 1, N), np.float32)
        # own query half first (static residual slice), other half after
        xf_aug[:C, :NI] = xf3[b][:, ih * NI : (ih + 1) * NI]
        xf_aug[:C, NI:] = xf3[b][:, (1 - ih) * NI : (2 - ih) * NI]
        xf_aug[C] = 1.0
        in_maps.append(
            {
                "xf": xf_aug.astype(ml_dtypes.bfloat16),
                "xq": np.ascontiguousarray(
                    xr3[b][:, ih * NI : (ih + 1) * NI]
                ).astype(ml_dtypes.bfloat16),
                "cb": cblob.astype(ml_dtypes.bfloat16),
            }
        )

    from concourse.bass_utils import run_bass_kernel_spmd

    res = run_bass_kernel_spmd(nc, in_maps, list(range(NCORES)))

    out = np.empty((B, C, N), np.float32)
    for core in range(NCORES):
        b, ih = core >> 1, core & 1
        out[b][:, ih * NI : (ih + 1) * NI] = res.results[core]["y"]
    return out.reshape(B, C, HH, WW)


# revision 20
# speedup vs baseline: 1.5736x; 1.0759x over previous
"""Trainium2 Bass kernel for the MFPA attention module.

Reference computation (per batch b, with N = H*W = 4096 spatial sites):
    q = Wq @ x_RGB + bq            (CQK=16 channels)
    k = Wk @ x    + bk
    v = Wv @ x    + bv             (C=64 channels)
    energy[i,j] = q_i . k_j
    att = softmax(energy, axis=j)
    out[c,i] = sum_j v[c,j] att[i,j]
    y = lam * out + x

Device strategy (8 NeuronCores): data-parallel over batch (4) x query-row
halves (2).  Each core holds x[b] fully (for K/V and the residual) and its
2048-row query slice, and computes a flash-style streaming softmax so the
4096x4096 energy matrix never leaves PSUM/SBUF.

Host-side weight folding (softmax is shift-invariant, so bk drops out):
    energy[i,j] = (M^T xr_i + bqk) . xf_j    with  M = Wq^T Wk, bqk = Wk^T bq

v2 pipeline notes:
  * exp of the energy is split across BOTH the Scalar (true Exp -> fp8e4)
    and Vector engines (exponent-stuffing: byte = clamp(K8*e + 40) is the
    fp8e4 bit pattern of exp(e)/4, computed by one tensor_scalar).  A
    constant 40/K8 is folded into the energy via the ones row of xf_aug
    (extra qk channel), so the DVE op needs only (mult, max).
  * attention weights p and v are fp8e4; the PV matmul runs DoubleRow
    (256-deep contraction), halving tensor-engine streaming time.
  * the 1/rowsum uses the fast custom-DVE reciprocal; lam and a fp8
    range scale S are folded into wv on the host.
  * residual is taken from the bf16 xf tile (xres input dropped).
  * a short burst of dummy matmuls warms the PE HAM clock-gate while the
    input DMAs stream.
"""

import ml_dtypes
import numpy as np

import concourse.bass as bass
import concourse.mybir as mybir
import concourse.tile as tile_mod
from concourse.vector_clock import ScopedClock

B, C, HH, WW = 4, 64, 64, 64
N = HH * WW          # 4096 spatial sites
NI = N // 2          # query rows per core
CHUNK = 512          # query rows processed per main-loop iteration
NCHUNK = NI // CHUNK
JBLK = 128           # key/value block (PSUM partition dim)
NJP = N // (2 * JBLK)  # 16 j-pair groups (256 keys each)
NCORES = 8
NWARM = 10           # HAM warm-up matmuls

F32 = mybir.dt.float32
F32R = mybir.dt.float32r
BF16 = mybir.dt.bfloat16
F8 = mybir.dt.float8e4
U8 = mybir.dt.uint8
I32 = mybir.dt.int32

K8 = 8.0 / float(np.log(2.0))       # 11.5416  (fp8e4 octave = 8 bytes)
BIAS_B = 40.0                       # byte bias; fp8e4 exp bias is 56 -> p = exp(e)/4
BIAS_ROW = BIAS_B / K8              # energy offset carried by the qk ones-channel
ACT_BIAS = -(BIAS_ROW + float(2.0 * np.log(2.0)))   # Exp(e' + ACT_BIAS) = exp(e)/4
VSCALE = 16.0                       # fp8 range scale on v (cancels in p@v / p@1)
# exp on the vector engine for these j-blocks (14 of 32; DVE also carries
# the bit-trick reciprocal + epilogue muls), scalar takes 18
DVE_BLOCKS = frozenset((1, 3, 5, 7, 9, 11, 13, 19, 21, 23, 25, 27, 29, 31))
RMAGIC = float(0x7EF31000)          # bits(1/x) ~= RMAGIC - bits(x), ~5% max err
NWARM2 = 24


def _patched_drain_and_barrier(self, tick_clock, wait_clock):
    # The walrus build in this container rejects instructions with more than
    # one sync-wait command ("Too many sync wait commands" on the Tile tail
    # drain).  Split the aggregated drain into one drain per semaphore wait.
    nc = self.nc
    drain_inst = nc.sync.drain()
    wait_clock.add_sem_waits(
        drain_inst.ins, ScopedClock({None: tick_clock.global_clock})
    )
    inst = drain_inst.ins
    si = inst.sync_info
    waits = list(si.on_wait or []) if si else []
    if len(waits) > 1:
        si.on_wait = waits[:1]
        for w in waits[1:]:
            extra = nc.sync.drain()
            extra.ins.sync_info = mybir.SyncInfo(on_wait=[w], on_update=[])
    nc.all_engine_barrier()
    popped = nc._tile_sem_poison_stack.pop()
    assert popped is self._sem_poison
    nc.clear_and_free_semaphores(list(self.sems.allocated().values()))
    nc.all_engine_barrier()


tile_mod.TileContext._drain_and_barrier = _patched_drain_and_barrier


def _split_multi_waits(nc):
    # This walrus build accepts at most one sync-wait command per TPB
    # instruction.  Hoist extra waits onto engine NoOps placed just before
    # the instruction (engine executes in order, so semantics are kept).
    for blk in nc.m.functions[0].blocks:
        insts = list(blk.instructions)
        out = []
        changed = False
        for inst in insts:
            si = inst.sync_info
            if si is not None and si.on_wait and len(si.on_wait) > 1:
                waits = list(si.on_wait)
                si.on_wait = waits[-1:]
                for w in waits[:-1]:
                    nop = mybir.InstNoOp(name=nc.get_next_instruction_name())
                    nop.engine = inst.engine
                    nop.sync_info = mybir.SyncInfo(on_wait=[w], on_update=[])
                    out.append(nop)
                changed = True
            out.append(inst)
        if changed:
            blk.instructions = out


def build_bass(split_waits=True):
    nc = bass.Bass()
    xf = nc.declare_dram_parameter("xf", [C + 1, N], BF16, isOutput=False)
    xq = nc.declare_dram_parameter("xq", [C, NI], BF16, isOutput=False)
    # consts blob: cols 0:64 = M, col 64 = zero, col 65 = bias, cols 66:132 = wv_aug
    cb = nc.declare_dram_parameter("cb", [128, 132], BF16, isOutput=False)
    y = nc.declare_dram_parameter("y", [C, NI], F32, isOutput=True)

    with tile_mod.TileContext(nc) as tc:
        with (
            tc.tile_pool(name="singles", bufs=1) as singles,
            tc.tile_pool(name="ppool", bufs=6) as ppool,
            tc.tile_pool(name="ypool", bufs=3) as ypool,
            tc.tile_pool(name="lpool", bufs=2) as lpool,
            tc.tile_pool(name="rpool", bufs=2) as rpool,
            tc.tile_pool(name="ps_et", bufs=4, space="PSUM") as ps_et,
            tc.tile_pool(name="ps_pv", bufs=2, space="PSUM") as ps_pv,
            tc.tile_pool(name="ps_misc", bufs=2, space="PSUM") as ps_misc,
        ):
            # ---- tiny SBUF constants (no DMA) -----------------------------
            wsb = singles.tile([128, CHUNK], BF16)
            nc.vector.memset(wsb, 0.0)
            ones_sb = singles.tile([1, C], BF16)
            nc.vector.memset(ones_sb, 1.0)
            actb_sb = singles.tile([128, 1], F32)
            nc.vector.memset(actb_sb, ACT_BIAS)
            # the first ACTIVATE pays the ~2.7us exp table load; fire it on a
            # dummy tile immediately so it overlaps the input DMAs/warm-up
            tblw = singles.tile([128, 1], F32)
            nc.scalar.activation(
                out=tblw, in_=actb_sb,
                func=mybir.ActivationFunctionType.Exp,
            )

            # ---- input DMAs: few triggers, spread across the three DMA-
            # capable queues (sync / gpsimd / scalar) for parallel rings
            xq_sb = singles.tile([C, NI], BF16)
            nc.scalar.dma_start(out=xq_sb, in_=xq[:, :])
            cb_sb = singles.tile([128, 132], BF16)
            nc.gpsimd.dma_start(out=cb_sb, in_=cb[:, :])
            # K=128 contraction everywhere (the fast weight-load path needs
            # full-height weights); rows C+1..127 are zero on the qk side.
            xf_sb = singles.tile([128, N], BF16)
            nc.vector.memset(xf_sb[64:128, :], 0.0)
            NQ = N // 4
            nc.sync.dma_start(out=xf_sb[0 : C + 1, 0:NQ], in_=xf[:, 0:NQ])
            nc.gpsimd.dma_start(
                out=xf_sb[0 : C + 1, NQ : 2 * NQ], in_=xf[:, NQ : 2 * NQ]
            )
            nc.sync.dma_start(
                out=xf_sb[0 : C + 1, 2 * NQ : 3 * NQ], in_=xf[:, 2 * NQ : 3 * NQ]
            )
            nc.scalar.dma_start(
                out=xf_sb[0 : C + 1, 3 * NQ : N], in_=xf[:, 3 * NQ : N]
            )
            # xf arrives column-permuted so this core's own query half sits
            # at columns 0:NI (softmax is j-permutation invariant); row C is
            # all-ones: it feeds the V bias row, the qk energy-bias channel
            # and the fp8 rowsum column.

            # ---- HAM warm-up: a gapless accumulating matmul burst latches
            # the PE clock-gate to full speed while the input DMAs stream
            wp = ps_misc.tile([128, CHUNK], F32, tag="misc")
            for k in range(NWARM2):
                nc.tensor.matmul(
                    out=wp, lhsT=wsb[:, 0:128], rhs=wsb,
                    start=(k == 0), stop=(k == NWARM2 - 1),
                )

            bqk_sb = singles.tile([C + 1, 1], F32)
            nc.vector.tensor_copy(bqk_sb, cb_sb[0 : C + 1, 65:66])

            # ---- Q.K preparation -----------------------------------------
            # qs row C is 0 (M blob column 64..65 region row C is zero), and
            # the bias vector carries BIAS_ROW there, so the Identity
            # activation fills the whole (C+1)-row qk tile in one op.
            # Only chunk 0 is prepared up front; chunk ic+1's qk is produced
            # inside chunk ic's block stream so the PE never idles on it.
            qk_sbs = [
                singles.tile([128, CHUNK], BF16, name=f"qk_sb{ic}")
                for ic in range(NCHUNK)
            ]
            for t in qk_sbs:
                nc.vector.memset(t[64:128, :], 0.0)

            def emit_qk_prep(ic):
                isl = slice(ic * CHUNK, (ic + 1) * CHUNK)
                qs = ps_misc.tile([C + 1, CHUNK], F32, tag="misc")
                nc.tensor.matmul(
                    out=qs, lhsT=cb_sb[0:C, 0:65], rhs=xq_sb[:, isl],
                    start=True, stop=True,
                )
                nc.scalar.activation(
                    out=qk_sbs[ic][0 : C + 1, :], in_=qs,
                    func=mybir.ActivationFunctionType.Identity,
                    bias=bqk_sb[:, 0:1], scale=1.0,
                )

            emit_qk_prep(0)

            # ---- V preparation: fp8 v_pair[jp, pair, t, c] ----------------
            # pair-dim byte step must be 16-aligned for DoubleRow ldweights -> pad 66 to 80
            v_sb = singles.tile([JBLK, NJP, 2, 80], F8)
            for m4 in range(N // (4 * JBLK)):
                vp = ps_misc.tile([JBLK, 2, 2, 66], F32, tag="misc")
                for k in range(4):
                    jb = 4 * m4 + k
                    nc.tensor.matmul(
                        out=vp[:, k // 2, k % 2, :],
                        lhsT=xf_sb[:, jb * JBLK : (jb + 1) * JBLK],
                        rhs=cb_sb[0:128, 66:132],
                        start=True, stop=True,
                    )
                nc.scalar.activation(
                    out=v_sb[:, 2 * m4 : 2 * m4 + 2, :, 0:66], in_=vp,
                    func=mybir.ActivationFunctionType.Copy,
                )

            # ---- main loop over query chunks ------------------------------
            # Per chunk: 32 j-block energy matmuls -> exp (split ACT/DVE)
            # -> DoubleRow PV accumulation, software-pipelined so the PE
            # never waits: PV of pair jp issues ~2 j-blocks after its exp,
            # and the previous chunk's epilogue tail (which trails the slow
            # reciprocal) is emitted a few groups into the next chunk.
            tail_ops = []
            pvq = []          # deferred cross-chunk PV tail + reciprocal
            for ic in range(NCHUNK):
                isl = slice(ic * CHUNK, (ic + 1) * CHUNK)
                qk_sb = qk_sbs[ic]
                pv = ps_pv.tile([66, CHUNK], F32)
                p_t = None
                for jb in range(2 * NJP):
                    et = ps_et.tile([JBLK, CHUNK], F32)
                    nc.tensor.matmul(
                        out=et,
                        lhsT=xf_sb[:, jb * JBLK : (jb + 1) * JBLK],
                        rhs=qk_sb,
                        start=True, stop=True,
                    )
                    if jb % 2 == 0:
                        p_t = ppool.tile([JBLK, 2, CHUNK], F8)
                        p_ts = [p_t] if jb == 0 else p_ts + [p_t]
                    if jb in DVE_BLOCKS:
                        # byte = clamp(round(K8*e + 40)) == fp8e4(exp(e)/4)
                        nc.vector.tensor_scalar(
                            out=p_t[:, jb % 2, :].bitcast(U8), in0=et[:, :],
                            scalar1=K8, scalar2=0.0,
                            op0=mybir.AluOpType.mult, op1=mybir.AluOpType.max,
                        )
                    else:
                        nc.scalar.activation(
                            out=p_t[:, jb % 2, :], in_=et[:, :],
                            func=mybir.ActivationFunctionType.Exp,
                            bias=actb_sb[:, 0:1], scale=1.0,
                        )
                    if jb in (1, 3) and pvq:
                        pvq.pop(0)()
                    if jb == 8 and tail_ops:
                        tail_ops.pop(0)()
                    if jb == 12 and ic + 1 < NCHUNK:
                        emit_qk_prep(ic + 1)
                    if jb >= 5 and (jb - 5) % 2 == 0:
                        jp = (jb - 5) // 2
                        nc.tensor.matmul(
                            out=pv[:, :], lhsT=v_sb[:, jp, :, 0:66],
                            rhs=p_ts[jp][:, :, :],
                            start=(jp == 0), stop=False,
                            perf_mode=mybir.MatmulPerfMode.DoubleRow,
                        )

                def make_pv_tail(pv=pv, p_ts=p_ts, jp=NJP - 2):
                    def emit():
                        nc.tensor.matmul(
                            out=pv[:, :], lhsT=v_sb[:, jp, :, 0:66],
                            rhs=p_ts[jp][:, :, :],
                            start=False, stop=(jp == NJP - 1),
                            perf_mode=mybir.MatmulPerfMode.DoubleRow,
                        )
                    return emit

                def make_recip(pv=pv, ic=ic, isl=isl):
                    def emit():
                        make_pv_tail(pv=pv, jp=NJP - 1)()
                        # exponent-flip bit-trick reciprocal of the rowsum
                        r0 = rpool.tile([1, CHUNK], I32)
                        nc.vector.tensor_scalar(
                            out=r0, in0=pv[64:65, :].bitcast(I32),
                            scalar1=-1.0, scalar2=RMAGIC,
                            op0=mybir.AluOpType.mult, op1=mybir.AluOpType.add,
                        )
                        r_bf = rpool.tile([1, CHUNK], BF16)
                        if ic == NCHUNK - 1:
                            nc.scalar.copy(r_bf, r0[:, :].bitcast(F32))
                        else:
                            nc.gpsimd.tensor_copy(r_bf, r0[:, :].bitcast(F32))
                        tail_ops.append(make_tail(pv, r_bf, isl, ic))
                    return emit

                def make_tail(pv, r_bf, isl, ic):
                    def emit():
                        lrb = ps_misc.tile([C, CHUNK], F32, tag="misc")
                        nc.tensor.matmul(
                            out=lrb, lhsT=ones_sb[:, :], rhs=r_bf[:, :],
                            start=True, stop=True,
                        )
                        lrb_sb = lpool.tile([C, CHUNK], F32)
                        nc.scalar.copy(lrb_sb, lrb)
                        y_t = ypool.tile([C, CHUNK], F32)
                        nc.vector.tensor_tensor(
                            out=y_t, in0=pv[0:C, :], in1=lrb_sb,
                            op=mybir.AluOpType.mult,
                        )
                        nc.vector.tensor_tensor(
                            out=y_t, in0=y_t, in1=xf_sb[0:C, isl],
                            op=mybir.AluOpType.add,
                        )
                        eng = nc.sync if ic % 2 == 0 else nc.gpsimd
                        eng.dma_start(out=y[:, isl], in_=y_t)
                    return emit

                pvq = [make_pv_tail(), make_recip()]
            for fn in pvq:
                fn()
            for fn in tail_ops:
                fn()

    if split_waits:
        _split_multi_waits(nc)
    return nc


_CACHE = {}


def kernel(**inputs):
    x = np.ascontiguousarray(np.asarray(inputs["x"], dtype=np.float32))
    x_RGB = np.ascontiguousarray(np.asarray(inputs["x_RGB"], dtype=np.float32))
    Wq = np.asarray(inputs["Wq"], dtype=np.float32)
    bq = np.asarray(inputs["bq"], dtype=np.float32)
    Wk = np.asarray(inputs["Wk"], dtype=np.float32)
    Wv = np.asarray(inputs["Wv"], dtype=np.float32)
    bv = np.asarray(inputs["bv"], dtype=np.float32)
    lam = np.asarray(inputs["lam"], dtype=np.float32)

    M = (Wq.T.astype(np.float64) @ Wk.astype(np.float64)).astype(np.float32)
    bqk = (Wk.T.astype(np.float64) @ bq.astype(np.float64)).astype(np.float32)

    ls = float(lam.reshape(-1)[0]) * VSCALE
    wv_aug = np.zeros((C + 1, 66), np.float32)
    wv_aug[:C, :C] = Wv.T * ls
    wv_aug[C, :C] = bv * ls
    wv_aug[C, 64] = VSCALE

    cblob = np.zeros((128, 132), np.float32)
    cblob[0:C, 0:64] = M
    cblob[0:C, 65] = bqk
    cblob[C, 65] = BIAS_ROW
    cblob[0 : C + 1, 66:132] = wv_aug

    xf3 = x.reshape(B, C, N)
    xr3 = x_RGB.reshape(B, C, N)

    if "nc" not in _CACHE:
        _CACHE["nc"] = build_bass()
    nc = _CACHE["nc"]

    in_maps = []
    for core in range(NCORES):
        b, ih = core >> 1, core & 1
        xf_aug = np.empty((C + 1, N), np.float32)
        # own query half first (static residual slice), other half after
        xf_aug[:C, :NI] = xf3[b][:, ih * NI : (ih + 1) * NI]
        xf_aug[:C, NI:] = xf3[b][:, (1 - ih) * NI : (2 - ih) * NI]
        xf_aug[C] = 1.0
        in_maps.append(
            {
                "xf": xf_aug.astype(ml_dtypes.bfloat16),
                "xq": np.ascontiguousarray(
                    xr3[b][:, ih * NI : (ih + 1) * NI]
                ).astype(ml_dtypes.bfloat16),
                "cb": cblob.astype(ml_dtypes.bfloat16),
            }
        )

    from concourse.bass_utils import run_bass_kernel_spmd

    res = run_bass_kernel_spmd(nc, in_maps, list(range(NCORES)))

    out = np.empty((B, C, N), np.float32)
    for core in range(NCORES):
        b, ih = core >> 1, core & 1
        out[b][:, ih * NI : (ih + 1) * NI] = res.results[core]["y"]
    return out.reshape(B, C, HH, WW)


# revision 21
# speedup vs baseline: 1.6461x; 1.0460x over previous
"""Trainium2 Bass kernel for the MFPA attention module.

Reference computation (per batch b, with N = H*W = 4096 spatial sites):
    q = Wq @ x_RGB + bq            (CQK=16 channels)
    k = Wk @ x    + bk
    v = Wv @ x    + bv             (C=64 channels)
    energy[i,j] = q_i . k_j
    att = softmax(energy, axis=j)
    out[c,i] = sum_j v[c,j] att[i,j]
    y = lam * out + x

Device strategy (8 NeuronCores): data-parallel over batch (4) x query-row
halves (2).  Each core holds x[b] fully (for K/V and the residual) and its
2048-row query slice, and computes a flash-style streaming softmax so the
4096x4096 energy matrix never leaves PSUM/SBUF.

Host-side weight folding (softmax is shift-invariant, so bk drops out):
    energy[i,j] = (M^T xr_i + bqk) . xf_j    with  M = Wq^T Wk, bqk = Wk^T bq

v2 pipeline notes:
  * exp of the energy is split across BOTH the Scalar (true Exp -> fp8e4)
    and Vector engines (exponent-stuffing: byte = clamp(K8*e + 40) is the
    fp8e4 bit pattern of exp(e)/4, computed by one tensor_scalar).  A
    constant 40/K8 is folded into the energy via the ones row of xf_aug
    (extra qk channel), so the DVE op needs only (mult, max).
  * attention weights p and v are fp8e4; the PV matmul runs DoubleRow
    (256-deep contraction), halving tensor-engine streaming time.
  * the 1/rowsum uses the fast custom-DVE reciprocal; lam and a fp8
    range scale S are folded into wv on the host.
  * residual is taken from the bf16 xf tile (xres input dropped).
  * a short burst of dummy matmuls warms the PE HAM clock-gate while the
    input DMAs stream.
"""

import ml_dtypes
import numpy as np

import concourse.bass as bass
import concourse.mybir as mybir
import concourse.tile as tile_mod
from concourse.vector_clock import ScopedClock

B, C, HH, WW = 4, 64, 64, 64
N = HH * WW          # 4096 spatial sites
NI = N // 2          # query rows per core
CHUNK = 512          # query rows processed per main-loop iteration
NCHUNK = NI // CHUNK
JBLK = 128           # key/value block (PSUM partition dim)
NJP = N // (2 * JBLK)  # 16 j-pair groups (256 keys each)
NCORES = 8
NWARM = 10           # HAM warm-up matmuls

F32 = mybir.dt.float32
F32R = mybir.dt.float32r
BF16 = mybir.dt.bfloat16
F8 = mybir.dt.float8e4
U8 = mybir.dt.uint8
I32 = mybir.dt.int32

K8 = 8.0 / float(np.log(2.0))       # 11.5416  (fp8e4 octave = 8 bytes)
BIAS_B = 40.0                       # byte bias; fp8e4 exp bias is 56 -> p = exp(e)/4
BIAS_ROW = BIAS_B / K8              # energy offset carried by the qk ones-channel
ACT_BIAS = -(BIAS_ROW + float(2.0 * np.log(2.0)))   # Exp(e' + ACT_BIAS) = exp(e)/4
VSCALE = 16.0                       # fp8 range scale on v (cancels in p@v / p@1)
# exp on the vector engine for these j-blocks (14 of 32; DVE also carries
# the bit-trick reciprocal + epilogue muls), scalar takes 18
DVE_BLOCKS = frozenset(j for j in range(1, 32, 2) if j != 15)
RMAGIC = float(0x7EF31000)          # bits(1/x) ~= RMAGIC - bits(x), ~5% max err
NWARM2 = 24


def _patched_drain_and_barrier(self, tick_clock, wait_clock):
    # The walrus build in this container rejects instructions with more than
    # one sync-wait command ("Too many sync wait commands" on the Tile tail
    # drain).  Split the aggregated drain into one drain per semaphore wait.
    nc = self.nc
    drain_inst = nc.sync.drain()
    wait_clock.add_sem_waits(
        drain_inst.ins, ScopedClock({None: tick_clock.global_clock})
    )
    inst = drain_inst.ins
    si = inst.sync_info
    waits = list(si.on_wait or []) if si else []
    if len(waits) > 1:
        si.on_wait = waits[:1]
        for w in waits[1:]:
            extra = nc.sync.drain()
            extra.ins.sync_info = mybir.SyncInfo(on_wait=[w], on_update=[])
    nc.all_engine_barrier()
    popped = nc._tile_sem_poison_stack.pop()
    assert popped is self._sem_poison
    nc.clear_and_free_semaphores(list(self.sems.allocated().values()))
    nc.all_engine_barrier()


tile_mod.TileContext._drain_and_barrier = _patched_drain_and_barrier


def _split_multi_waits(nc):
    # This walrus build accepts at most one sync-wait command per TPB
    # instruction.  Hoist extra waits onto engine NoOps placed just before
    # the instruction (engine executes in order, so semantics are kept).
    for blk in nc.m.functions[0].blocks:
        insts = list(blk.instructions)
        out = []
        changed = False
        for inst in insts:
            si = inst.sync_info
            if si is not None and si.on_wait and len(si.on_wait) > 1:
                waits = list(si.on_wait)
                si.on_wait = waits[-1:]
                for w in waits[:-1]:
                    nop = mybir.InstNoOp(name=nc.get_next_instruction_name())
                    nop.engine = inst.engine
                    nop.sync_info = mybir.SyncInfo(on_wait=[w], on_update=[])
                    out.append(nop)
                changed = True
            out.append(inst)
        if changed:
            blk.instructions = out


def build_bass(split_waits=True):
    nc = bass.Bass()
    xf = nc.declare_dram_parameter("xf", [C + 1, N], BF16, isOutput=False)
    xq = nc.declare_dram_parameter("xq", [C, NI], BF16, isOutput=False)
    # consts blob: cols 0:64 = M, col 64 = zero, col 65 = bias, cols 66:132 = wv_aug
    cb = nc.declare_dram_parameter("cb", [128, 132], BF16, isOutput=False)
    y = nc.declare_dram_parameter("y", [C, NI], F32, isOutput=True)

    with tile_mod.TileContext(nc) as tc:
        with (
            tc.tile_pool(name="singles", bufs=1) as singles,
            tc.tile_pool(name="ppool", bufs=6) as ppool,
            tc.tile_pool(name="ypool", bufs=3) as ypool,
            tc.tile_pool(name="lpool", bufs=2) as lpool,
            tc.tile_pool(name="rpool", bufs=2) as rpool,
            tc.tile_pool(name="ps_et", bufs=5, space="PSUM") as ps_et,
            tc.tile_pool(name="ps_pv", bufs=2, space="PSUM") as ps_pv,
            tc.tile_pool(name="ps_misc", bufs=1, space="PSUM") as ps_misc,
        ):
            # ---- tiny SBUF constants (no DMA) -----------------------------
            wsb = singles.tile([128, CHUNK], BF16)
            nc.vector.memset(wsb, 0.0)
            ones_sb = singles.tile([1, C], BF16)
            nc.vector.memset(ones_sb, 1.0)
            actb_sb = singles.tile([128, 1], F32)
            nc.vector.memset(actb_sb, ACT_BIAS)
            # the first ACTIVATE pays the ~2.7us exp table load; fire it on a
            # dummy tile immediately so it overlaps the input DMAs/warm-up
            tblw = singles.tile([128, 1], F32)
            nc.scalar.activation(
                out=tblw, in_=actb_sb,
                func=mybir.ActivationFunctionType.Exp,
            )

            # ---- input DMAs: few triggers, spread across the three DMA-
            # capable queues (sync / gpsimd / scalar) for parallel rings
            xq_sb = singles.tile([C, NI], BF16)
            nc.scalar.dma_start(out=xq_sb, in_=xq[:, :])
            cb_sb = singles.tile([128, 132], BF16)
            nc.gpsimd.dma_start(out=cb_sb, in_=cb[:, :])
            # K=128 contraction everywhere (the fast weight-load path needs
            # full-height weights); rows C+1..127 are zero on the qk side.
            xf_sb = singles.tile([128, N], BF16)
            nc.vector.memset(xf_sb[64:128, :], 0.0)
            NQ = N // 4
            nc.sync.dma_start(out=xf_sb[0 : C + 1, 0:NQ], in_=xf[:, 0:NQ])
            nc.gpsimd.dma_start(
                out=xf_sb[0 : C + 1, NQ : 2 * NQ], in_=xf[:, NQ : 2 * NQ]
            )
            nc.sync.dma_start(
                out=xf_sb[0 : C + 1, 2 * NQ : 3 * NQ], in_=xf[:, 2 * NQ : 3 * NQ]
            )
            nc.scalar.dma_start(
                out=xf_sb[0 : C + 1, 3 * NQ : N], in_=xf[:, 3 * NQ : N]
            )
            # xf arrives column-permuted so this core's own query half sits
            # at columns 0:NI (softmax is j-permutation invariant); row C is
            # all-ones: it feeds the V bias row, the qk energy-bias channel
            # and the fp8 rowsum column.

            # ---- HAM warm-up: a gapless accumulating matmul burst latches
            # the PE clock-gate to full speed while the input DMAs stream
            wp = ps_misc.tile([128, CHUNK], F32, tag="misc")
            for k in range(NWARM2):
                nc.tensor.matmul(
                    out=wp, lhsT=wsb[:, 0:128], rhs=wsb,
                    start=(k == 0), stop=(k == NWARM2 - 1),
                )

            bqk_sb = singles.tile([C + 1, 1], F32)
            nc.vector.tensor_copy(bqk_sb, cb_sb[0 : C + 1, 65:66])

            # ---- Q.K preparation -----------------------------------------
            # qs row C is 0 (M blob column 64..65 region row C is zero), and
            # the bias vector carries BIAS_ROW there, so the Identity
            # activation fills the whole (C+1)-row qk tile in one op.
            # Only chunk 0 is prepared up front; chunk ic+1's qk is produced
            # inside chunk ic's block stream so the PE never idles on it.
            qk_sbs = [
                singles.tile([128, CHUNK], BF16, name=f"qk_sb{ic}")
                for ic in range(NCHUNK)
            ]
            for t in qk_sbs:
                nc.vector.memset(t[64:128, :], 0.0)

            def emit_qk_prep(ic):
                isl = slice(ic * CHUNK, (ic + 1) * CHUNK)
                qs = ps_misc.tile([C + 1, CHUNK], F32, tag="misc")
                nc.tensor.matmul(
                    out=qs, lhsT=cb_sb[0:C, 0:65], rhs=xq_sb[:, isl],
                    start=True, stop=True,
                )
                nc.scalar.activation(
                    out=qk_sbs[ic][0 : C + 1, :], in_=qs,
                    func=mybir.ActivationFunctionType.Identity,
                    bias=bqk_sb[:, 0:1], scale=1.0,
                )

            emit_qk_prep(0)

            # ---- V preparation: fp8 v_pair[jp, pair, t, c] ----------------
            # pair-dim byte step must be 16-aligned for DoubleRow ldweights -> pad 66 to 80
            v_sb = singles.tile([JBLK, NJP, 2, 80], F8)
            for m4 in range(N // (4 * JBLK)):
                vp = ps_misc.tile([JBLK, 2, 2, 66], F32, tag="misc")
                for k in range(4):
                    jb = 4 * m4 + k
                    nc.tensor.matmul(
                        out=vp[:, k // 2, k % 2, :],
                        lhsT=xf_sb[:, jb * JBLK : (jb + 1) * JBLK],
                        rhs=cb_sb[0:128, 66:132],
                        start=True, stop=True,
                    )
                nc.scalar.activation(
                    out=v_sb[:, 2 * m4 : 2 * m4 + 2, :, 0:66], in_=vp,
                    func=mybir.ActivationFunctionType.Copy,
                )

            # ---- main loop over query chunks ------------------------------
            # Per chunk: 32 j-block energy matmuls -> exp (split ACT/DVE)
            # -> DoubleRow PV accumulation, software-pipelined so the PE
            # never waits: PV of pair jp issues ~2 j-blocks after its exp,
            # and the previous chunk's epilogue tail (which trails the slow
            # reciprocal) is emitted a few groups into the next chunk.
            tail_ops = []
            pvq = []          # deferred cross-chunk PV tail + reciprocal
            for ic in range(NCHUNK):
                isl = slice(ic * CHUNK, (ic + 1) * CHUNK)
                qk_sb = qk_sbs[ic]
                pv = ps_pv.tile([66, CHUNK], F32)
                p_t = None
                for jb in range(2 * NJP):
                    et = ps_et.tile([JBLK, CHUNK], F32)
                    nc.tensor.matmul(
                        out=et,
                        lhsT=xf_sb[:, jb * JBLK : (jb + 1) * JBLK],
                        rhs=qk_sb,
                        start=True, stop=True,
                    )
                    if jb % 2 == 0:
                        p_t = ppool.tile([JBLK, 2, CHUNK], F8)
                        p_ts = [p_t] if jb == 0 else p_ts + [p_t]
                    if jb in DVE_BLOCKS:
                        # byte = clamp(round(K8*e + 40)) == fp8e4(exp(e)/4)
                        nc.vector.tensor_scalar(
                            out=p_t[:, jb % 2, :].bitcast(U8), in0=et[:, :],
                            scalar1=K8, scalar2=0.0,
                            op0=mybir.AluOpType.mult, op1=mybir.AluOpType.max,
                        )
                    else:
                        nc.scalar.activation(
                            out=p_t[:, jb % 2, :], in_=et[:, :],
                            func=mybir.ActivationFunctionType.Exp,
                            bias=actb_sb[:, 0:1], scale=1.0,
                        )
                    if jb in (1, 3) and pvq:
                        pvq.pop(0)()
                    if jb == 8 and tail_ops:
                        tail_ops.pop(0)()
                    if jb == 12 and ic + 1 < NCHUNK:
                        emit_qk_prep(ic + 1)
                    if jb >= 5 and (jb - 5) % 2 == 0:
                        jp = (jb - 5) // 2
                        nc.tensor.matmul(
                            out=pv[:, :], lhsT=v_sb[:, jp, :, 0:66],
                            rhs=p_ts[jp][:, :, :],
                            start=(jp == 0), stop=False,
                            perf_mode=mybir.MatmulPerfMode.DoubleRow,
                        )

                def make_pv_tail(pv=pv, p_ts=p_ts, jp=NJP - 2):
                    def emit():
                        nc.tensor.matmul(
                            out=pv[:, :], lhsT=v_sb[:, jp, :, 0:66],
                            rhs=p_ts[jp][:, :, :],
                            start=False, stop=(jp == NJP - 1),
                            perf_mode=mybir.MatmulPerfMode.DoubleRow,
                        )
                    return emit

                def make_recip(pv=pv, ic=ic, isl=isl):
                    def emit():
                        make_pv_tail(pv=pv, jp=NJP - 1)()
                        # exponent-flip bit-trick reciprocal of the rowsum
                        r0 = rpool.tile([1, CHUNK], I32)
                        nc.vector.tensor_scalar(
                            out=r0, in0=pv[64:65, :].bitcast(I32),
                            scalar1=-1.0, scalar2=RMAGIC,
                            op0=mybir.AluOpType.mult, op1=mybir.AluOpType.add,
                        )
                        r_bf = rpool.tile([1, CHUNK], BF16)
                        if ic == NCHUNK - 1:
                            nc.scalar.copy(r_bf, r0[:, :].bitcast(F32))
                        else:
                            nc.gpsimd.tensor_copy(r_bf, r0[:, :].bitcast(F32))
                        tail_ops.append(make_tail(pv, r_bf, isl, ic))
                    return emit

                def make_tail(pv, r_bf, isl, ic):
                    def emit():
                        lrb = ps_misc.tile([C, CHUNK], F32, tag="misc")
                        nc.tensor.matmul(
                            out=lrb, lhsT=ones_sb[:, :], rhs=r_bf[:, :],
                            start=True, stop=True,
                        )
                        lrb_sb = lpool.tile([C, CHUNK], F32)
                        nc.scalar.copy(lrb_sb, lrb)
                        y_t = ypool.tile([C, CHUNK], F32)
                        nc.vector.tensor_tensor(
                            out=y_t, in0=pv[0:C, :], in1=lrb_sb,
                            op=mybir.AluOpType.mult,
                        )
                        nc.vector.tensor_tensor(
                            out=y_t, in0=y_t, in1=xf_sb[0:C, isl],
                            op=mybir.AluOpType.add,
                        )
                        eng = nc.sync if ic % 2 == 0 else nc.gpsimd
                        eng.dma_start(out=y[:, isl], in_=y_t)
                    return emit

                pvq = [make_pv_tail(), make_recip()]
            for fn in pvq:
                fn()
            for fn in tail_ops:
                fn()

    if split_waits:
        _split_multi_waits(nc)
    return nc


_CACHE = {}


def kernel(**inputs):
    x = np.ascontiguousarray(np.asarray(inputs["x"], dtype=np.float32))
    x_RGB = np.ascontiguousarray(np.asarray(inputs["x_RGB"], dtype=np.float32))
    Wq = np.asarray(inputs["Wq"], dtype=np.float32)
    bq = np.asarray(inputs["bq"], dtype=np.float32)
    Wk = np.asarray(inputs["Wk"], dtype=np.float32)
    Wv = np.asarray(inputs["Wv"], dtype=np.float32)
    bv = np.asarray(inputs["bv"], dtype=np.float32)
    lam = np.asarray(inputs["lam"], dtype=np.float32)

    M = (Wq.T.astype(np.float64) @ Wk.astype(np.float64)).astype(np.float32)
    bqk = (Wk.T.astype(np.float64) @ bq.astype(np.float64)).astype(np.float32)

    ls = float(lam.reshape(-1)[0]) * VSCALE
    wv_aug = np.zeros((C + 1, 66), np.float32)
    wv_aug[:C, :C] = Wv.T * ls
    wv_aug[C, :C] = bv * ls
    wv_aug[C, 64] = VSCALE

    cblob = np.zeros((128, 132), np.float32)
    cblob[0:C, 0:64] = M
    cblob[0:C, 65] = bqk
    cblob[C, 65] = BIAS_ROW
    cblob[0 : C + 1, 66:132] = wv_aug

    xf3 = x.reshape(B, C, N)
    xr3 = x_RGB.reshape(B, C, N)

    if "nc" not in _CACHE:
        _CACHE["nc"] = build_bass()
    nc = _CACHE["nc"]

    in_maps = []
    for core in range(NCORES):
        b, ih = core >> 1, core & 1
        xf_aug = np.empty((C + 1, N), np.float32)
        # own query half first (static residual slice), other half after
        xf_aug[:C, :NI] = xf3[b][:, ih * NI : (ih + 1) * NI]
        xf_aug[:C, NI:] = xf3[b][:, (1 - ih) * NI : (2 - ih) * NI]
        xf_aug[C] = 1.0
        in_maps.append(
            {
                "xf": xf_aug.astype(ml_dtypes.bfloat16),
                "xq": np.ascontiguousarray(
                    xr3[b][:, ih * NI : (ih + 1) * NI]
                ).astype(ml_dtypes.bfloat16),
                "cb": cblob.astype(ml_dtypes.bfloat16),
            }
        )

    from concourse.bass_utils import run_bass_kernel_spmd

    res = run_bass_kernel_spmd(nc, in_maps, list(range(NCORES)))

    out = np.empty((B, C, N), np.float32)
    for core in range(NCORES):
        b, ih = core >> 1, core & 1
        out[b][:, ih * NI : (ih + 1) * NI] = res.results[core]["y"]
    return out.reshape(B, C, HH, WW)


# revision 22
# speedup vs baseline: 1.6823x; 1.0220x over previous
"""Trainium2 Bass kernel for the MFPA attention module.

Reference computation (per batch b, with N = H*W = 4096 spatial sites):
    q = Wq @ x_RGB + bq            (CQK=16 channels)
    k = Wk @ x    + bk
    v = Wv @ x    + bv             (C=64 channels)
    energy[i,j] = q_i . k_j
    att = softmax(energy, axis=j)
    out[c,i] = sum_j v[c,j] att[i,j]
    y = lam * out + x

Device strategy (8 NeuronCores): data-parallel over batch (4) x query-row
halves (2).  Each core holds x[b] fully (for K/V and the residual) and its
2048-row query slice, and computes a flash-style streaming softmax so the
4096x4096 energy matrix never leaves PSUM/SBUF.

Host-side weight folding (softmax is shift-invariant, so bk drops out):
    energy[i,j] = (M^T xr_i + bqk) . xf_j    with  M = Wq^T Wk, bqk = Wk^T bq

v2 pipeline notes:
  * exp of the energy is split across BOTH the Scalar (true Exp -> fp8e4)
    and Vector engines (exponent-stuffing: byte = clamp(K8*e + 40) is the
    fp8e4 bit pattern of exp(e)/4, computed by one tensor_scalar).  A
    constant 40/K8 is folded into the energy via the ones row of xf_aug
    (extra qk channel), so the DVE op needs only (mult, max).
  * attention weights p and v are fp8e4; the PV matmul runs DoubleRow
    (256-deep contraction), halving tensor-engine streaming time.
  * the 1/rowsum uses the fast custom-DVE reciprocal; lam and a fp8
    range scale S are folded into wv on the host.
  * residual is taken from the bf16 xf tile (xres input dropped).
  * a short burst of dummy matmuls warms the PE HAM clock-gate while the
    input DMAs stream.
"""

import ml_dtypes
import numpy as np

import concourse.bass as bass
import concourse.mybir as mybir
import concourse.tile as tile_mod
from concourse.vector_clock import ScopedClock

B, C, HH, WW = 4, 64, 64, 64
N = HH * WW          # 4096 spatial sites
NI = N // 2          # query rows per core
CHUNK = 512          # query rows processed per main-loop iteration
NCHUNK = NI // CHUNK
JBLK = 128           # key/value block (PSUM partition dim)
NJP = N // (2 * JBLK)  # 16 j-pair groups (256 keys each)
NCORES = 8
NWARM = 10           # HAM warm-up matmuls

F32 = mybir.dt.float32
F32R = mybir.dt.float32r
BF16 = mybir.dt.bfloat16
F8 = mybir.dt.float8e4
U8 = mybir.dt.uint8
I32 = mybir.dt.int32

K8 = 8.0 / float(np.log(2.0))       # 11.5416  (fp8e4 octave = 8 bytes)
BIAS_B = 40.0                       # byte bias; fp8e4 exp bias is 56 -> p = exp(e)/4
BIAS_ROW = BIAS_B / K8              # energy offset carried by the qk ones-channel
ACT_BIAS = -(BIAS_ROW + float(2.0 * np.log(2.0)))   # Exp(e' + ACT_BIAS) = exp(e)/4
VSCALE = 16.0                       # fp8 range scale on v (cancels in p@v / p@1)
# exp on the vector engine for these j-blocks (14 of 32; DVE also carries
# the bit-trick reciprocal + epilogue muls), scalar takes 18
DVE_BLOCKS = frozenset(j for j in range(1, 32, 2) if j != 15)
RMAGIC = float(0x7EF31000)          # bits(1/x) ~= RMAGIC - bits(x), ~5% max err
NWARM2 = 24


def _patched_drain_and_barrier(self, tick_clock, wait_clock):
    # The walrus build in this container rejects instructions with more than
    # one sync-wait command ("Too many sync wait commands" on the Tile tail
    # drain).  Split the aggregated drain into one drain per semaphore wait.
    nc = self.nc
    drain_inst = nc.sync.drain()
    wait_clock.add_sem_waits(
        drain_inst.ins, ScopedClock({None: tick_clock.global_clock})
    )
    inst = drain_inst.ins
    si = inst.sync_info
    waits = list(si.on_wait or []) if si else []
    if len(waits) > 1:
        si.on_wait = waits[:1]
        for w in waits[1:]:
            extra = nc.sync.drain()
            extra.ins.sync_info = mybir.SyncInfo(on_wait=[w], on_update=[])
    nc.all_engine_barrier()
    popped = nc._tile_sem_poison_stack.pop()
    assert popped is self._sem_poison
    nc.clear_and_free_semaphores(list(self.sems.allocated().values()))
    nc.all_engine_barrier()


tile_mod.TileContext._drain_and_barrier = _patched_drain_and_barrier


def _split_multi_waits(nc):
    # This walrus build accepts at most one sync-wait command per TPB
    # instruction.  Hoist extra waits onto engine NoOps placed just before
    # the instruction (engine executes in order, so semantics are kept).
    for blk in nc.m.functions[0].blocks:
        insts = list(blk.instructions)
        out = []
        changed = False
        for inst in insts:
            si = inst.sync_info
            if si is not None and si.on_wait and len(si.on_wait) > 1:
                waits = list(si.on_wait)
                si.on_wait = waits[-1:]
                for w in waits[:-1]:
                    nop = mybir.InstNoOp(name=nc.get_next_instruction_name())
                    nop.engine = inst.engine
                    nop.sync_info = mybir.SyncInfo(on_wait=[w], on_update=[])
                    out.append(nop)
                changed = True
            out.append(inst)
        if changed:
            blk.instructions = out


def build_bass(split_waits=True):
    nc = bass.Bass()
    xf = nc.declare_dram_parameter("xf", [C + 1, N], BF16, isOutput=False)
    xq = nc.declare_dram_parameter("xq", [C, NI], BF16, isOutput=False)
    # consts blob: cols 0:64 = M, col 64 = zero, col 65 = bias, cols 66:132 = wv_aug
    cb = nc.declare_dram_parameter("cb", [128, 132], BF16, isOutput=False)
    y = nc.declare_dram_parameter("y", [C, NI], F32, isOutput=True)

    with tile_mod.TileContext(nc) as tc:
        with (
            tc.tile_pool(name="singles", bufs=1) as singles,
            tc.tile_pool(name="ppool", bufs=6) as ppool,
            tc.tile_pool(name="ypool", bufs=3) as ypool,
            tc.tile_pool(name="lpool", bufs=2) as lpool,
            tc.tile_pool(name="rpool", bufs=2) as rpool,
            tc.tile_pool(name="ps_et", bufs=5, space="PSUM") as ps_et,
            tc.tile_pool(name="ps_pv", bufs=2, space="PSUM") as ps_pv,
            tc.tile_pool(name="ps_misc", bufs=1, space="PSUM") as ps_misc,
        ):
            # ---- tiny SBUF constants (no DMA) -----------------------------
            wsb = singles.tile([128, CHUNK], BF16)
            nc.vector.memset(wsb, 0.0)
            ones_sb = singles.tile([1, C], F32)
            nc.vector.memset(ones_sb, 1.0)
            actb_sb = singles.tile([128, 1], F32)
            nc.vector.memset(actb_sb, ACT_BIAS)
            # the first ACTIVATE pays the ~2.7us exp table load; fire it on a
            # dummy tile immediately so it overlaps the input DMAs/warm-up
            tblw = singles.tile([128, 1], F32)
            nc.scalar.activation(
                out=tblw, in_=actb_sb,
                func=mybir.ActivationFunctionType.Exp,
            )

            # ---- input DMAs: few triggers, spread across the three DMA-
            # capable queues (sync / gpsimd / scalar) for parallel rings
            xq_sb = singles.tile([C, NI], BF16)
            nc.scalar.dma_start(out=xq_sb, in_=xq[:, :])
            cb_sb = singles.tile([128, 132], BF16)
            nc.gpsimd.dma_start(out=cb_sb, in_=cb[:, :])
            # K=128 contraction everywhere (the fast weight-load path needs
            # full-height weights); rows C+1..127 are zero on the qk side.
            xf_sb = singles.tile([128, N], BF16)
            nc.vector.memset(xf_sb[64:128, :], 0.0)
            NQ = N // 4
            nc.sync.dma_start(out=xf_sb[0 : C + 1, 0:NQ], in_=xf[:, 0:NQ])
            nc.gpsimd.dma_start(
                out=xf_sb[0 : C + 1, NQ : 2 * NQ], in_=xf[:, NQ : 2 * NQ]
            )
            nc.sync.dma_start(
                out=xf_sb[0 : C + 1, 2 * NQ : 3 * NQ], in_=xf[:, 2 * NQ : 3 * NQ]
            )
            nc.scalar.dma_start(
                out=xf_sb[0 : C + 1, 3 * NQ : N], in_=xf[:, 3 * NQ : N]
            )
            # xf arrives column-permuted so this core's own query half sits
            # at columns 0:NI (softmax is j-permutation invariant); row C is
            # all-ones: it feeds the V bias row, the qk energy-bias channel
            # and the fp8 rowsum column.

            # ---- HAM warm-up: a gapless accumulating matmul burst latches
            # the PE clock-gate to full speed while the input DMAs stream
            wp = ps_misc.tile([128, CHUNK], F32, tag="misc")
            for k in range(NWARM2):
                nc.tensor.matmul(
                    out=wp, lhsT=wsb[:, 0:128], rhs=wsb,
                    start=(k == 0), stop=(k == NWARM2 - 1),
                )

            bqk_sb = singles.tile([C + 1, 1], F32)
            nc.vector.tensor_copy(bqk_sb, cb_sb[0 : C + 1, 65:66])

            # ---- Q.K preparation -----------------------------------------
            # qs row C is 0 (M blob column 64..65 region row C is zero), and
            # the bias vector carries BIAS_ROW there, so the Identity
            # activation fills the whole (C+1)-row qk tile in one op.
            # Only chunk 0 is prepared up front; chunk ic+1's qk is produced
            # inside chunk ic's block stream so the PE never idles on it.
            qk_sbs = [
                singles.tile([128, CHUNK], BF16, name=f"qk_sb{ic}")
                for ic in range(NCHUNK)
            ]
            for t in qk_sbs:
                nc.vector.memset(t[64:128, :], 0.0)

            def emit_qk_prep(ic):
                isl = slice(ic * CHUNK, (ic + 1) * CHUNK)
                qs = ps_misc.tile([C + 1, CHUNK], F32, tag="misc")
                nc.tensor.matmul(
                    out=qs, lhsT=cb_sb[0:C, 0:65], rhs=xq_sb[:, isl],
                    start=True, stop=True,
                )
                nc.scalar.activation(
                    out=qk_sbs[ic][0 : C + 1, :], in_=qs,
                    func=mybir.ActivationFunctionType.Identity,
                    bias=bqk_sb[:, 0:1], scale=1.0,
                )

            emit_qk_prep(0)

            # ---- V preparation: fp8 v_pair[jp, pair, t, c] ----------------
            # pair-dim byte step must be 16-aligned for DoubleRow ldweights -> pad 66 to 80
            v_sb = singles.tile([JBLK, NJP, 2, 80], F8)
            for m4 in range(N // (4 * JBLK)):
                vp = ps_misc.tile([JBLK, 2, 2, 66], F32, tag="misc")
                for k in range(4):
                    jb = 4 * m4 + k
                    nc.tensor.matmul(
                        out=vp[:, k // 2, k % 2, :],
                        lhsT=xf_sb[:, jb * JBLK : (jb + 1) * JBLK],
                        rhs=cb_sb[0:128, 66:132],
                        start=True, stop=True,
                    )
                nc.scalar.activation(
                    out=v_sb[:, 2 * m4 : 2 * m4 + 2, :, 0:66], in_=vp,
                    func=mybir.ActivationFunctionType.Copy,
                )

            # ---- main loop over query chunks ------------------------------
            # Per chunk: 32 j-block energy matmuls -> exp (split ACT/DVE)
            # -> DoubleRow PV accumulation, software-pipelined so the PE
            # never waits: PV of pair jp issues ~2 j-blocks after its exp,
            # and the previous chunk's epilogue tail (which trails the slow
            # reciprocal) is emitted a few groups into the next chunk.
            tail_ops = []
            pvq = []          # deferred cross-chunk PV tail + reciprocal
            for ic in range(NCHUNK):
                isl = slice(ic * CHUNK, (ic + 1) * CHUNK)
                qk_sb = qk_sbs[ic]
                pv = ps_pv.tile([66, CHUNK], F32)
                p_t = None
                for jb in range(2 * NJP):
                    et = ps_et.tile([JBLK, CHUNK], F32)
                    nc.tensor.matmul(
                        out=et,
                        lhsT=xf_sb[:, jb * JBLK : (jb + 1) * JBLK],
                        rhs=qk_sb,
                        start=True, stop=True,
                    )
                    if jb % 2 == 0:
                        p_t = ppool.tile([JBLK, 2, CHUNK], F8)
                        p_ts = [p_t] if jb == 0 else p_ts + [p_t]
                    if jb in DVE_BLOCKS:
                        # byte = clamp(round(K8*e + 40)) == fp8e4(exp(e)/4)
                        nc.vector.tensor_scalar(
                            out=p_t[:, jb % 2, :].bitcast(U8), in0=et[:, :],
                            scalar1=K8, scalar2=0.0,
                            op0=mybir.AluOpType.mult, op1=mybir.AluOpType.max,
                        )
                    else:
                        nc.scalar.activation(
                            out=p_t[:, jb % 2, :], in_=et[:, :],
                            func=mybir.ActivationFunctionType.Exp,
                            bias=actb_sb[:, 0:1], scale=1.0,
                        )
                    if jb in (1, 3) and pvq:
                        pvq.pop(0)()
                    if jb == 8 and tail_ops:
                        tail_ops.pop(0)()
                    if jb == 12 and ic + 1 < NCHUNK:
                        emit_qk_prep(ic + 1)
                    if jb >= 5 and (jb - 5) % 2 == 0:
                        jp = (jb - 5) // 2
                        nc.tensor.matmul(
                            out=pv[:, :], lhsT=v_sb[:, jp, :, 0:66],
                            rhs=p_ts[jp][:, :, :],
                            start=(jp == 0), stop=False,
                            perf_mode=mybir.MatmulPerfMode.DoubleRow,
                        )

                def make_pv_tail(pv=pv, p_ts=p_ts, jp=NJP - 2):
                    def emit():
                        nc.tensor.matmul(
                            out=pv[:, :], lhsT=v_sb[:, jp, :, 0:66],
                            rhs=p_ts[jp][:, :, :],
                            start=False, stop=(jp == NJP - 1),
                            perf_mode=mybir.MatmulPerfMode.DoubleRow,
                        )
                    return emit

                def make_recip(pv=pv, ic=ic, isl=isl):
                    def emit():
                        make_pv_tail(pv=pv, jp=NJP - 1)()
                        # exponent-flip bit-trick reciprocal of the rowsum
                        r0 = rpool.tile([1, CHUNK], I32)
                        nc.vector.tensor_scalar(
                            out=r0, in0=pv[64:65, :].bitcast(I32),
                            scalar1=-1.0, scalar2=RMAGIC,
                            op0=mybir.AluOpType.mult, op1=mybir.AluOpType.add,
                        )
                        tail_ops.append(make_tail(pv, r0, isl, ic))
                    return emit

                def make_tail(pv, r0, isl, ic):
                    def emit():
                        # true-fp32 broadcast matmul reads the reciprocal
                        # bits directly (no conversion op on any engine)
                        lrb = ps_misc.tile([C, CHUNK], F32, tag="misc")
                        nc.tensor.matmul(
                            out=lrb, lhsT=ones_sb[:, :],
                            rhs=r0[:, :].bitcast(F32),
                            start=True, stop=True,
                        )
                        lrb_sb = lpool.tile([C, CHUNK], F32)
                        nc.scalar.copy(lrb_sb, lrb)
                        y_t = ypool.tile([C, CHUNK], F32)
                        nc.vector.tensor_tensor(
                            out=y_t, in0=pv[0:C, :], in1=lrb_sb,
                            op=mybir.AluOpType.mult,
                        )
                        nc.vector.tensor_tensor(
                            out=y_t, in0=y_t, in1=xf_sb[0:C, isl],
                            op=mybir.AluOpType.add,
                        )
                        eng = nc.sync if ic % 2 == 0 else nc.gpsimd
                        eng.dma_start(out=y[:, isl], in_=y_t)
                    return emit

                pvq = [make_pv_tail(), make_recip()]
            for fn in pvq:
                fn()
            for fn in tail_ops:
                fn()

    if split_waits:
        _split_multi_waits(nc)
    return nc


_CACHE = {}


def kernel(**inputs):
    x = np.ascontiguousarray(np.asarray(inputs["x"], dtype=np.float32))
    x_RGB = np.ascontiguousarray(np.asarray(inputs["x_RGB"], dtype=np.float32))
    Wq = np.asarray(inputs["Wq"], dtype=np.float32)
    bq = np.asarray(inputs["bq"], dtype=np.float32)
    Wk = np.asarray(inputs["Wk"], dtype=np.float32)
    Wv = np.asarray(inputs["Wv"], dtype=np.float32)
    bv = np.asarray(inputs["bv"], dtype=np.float32)
    lam = np.asarray(inputs["lam"], dtype=np.float32)

    M = (Wq.T.astype(np.float64) @ Wk.astype(np.float64)).astype(np.float32)
    bqk = (Wk.T.astype(np.float64) @ bq.astype(np.float64)).astype(np.float32)

    ls = float(lam.reshape(-1)[0]) * VSCALE
    wv_aug = np.zeros((C + 1, 66), np.float32)
    wv_aug[:C, :C] = Wv.T * ls
    wv_aug[C, :C] = bv * ls
    wv_aug[C, 64] = VSCALE

    cblob = np.zeros((128, 132), np.float32)
    cblob[0:C, 0:64] = M
    cblob[0:C, 65] = bqk
    cblob[C, 65] = BIAS_ROW
    cblob[0 : C + 1, 66:132] = wv_aug

    xf3 = x.reshape(B, C, N)
    xr3 = x_RGB.reshape(B, C, N)

    if "nc" not in _CACHE:
        _CACHE["nc"] = build_bass()
    nc = _CACHE["nc"]

    in_maps = []
    for core in range(NCORES):
        b, ih = core >> 1, core & 1
        xf_aug = np.empty((C + 1, N), np.float32)
        # own query half first (static residual slice), other half after
        xf_aug[:C, :NI] = xf3[b][:, ih * NI : (ih + 1) * NI]
        xf_aug[:C, NI:] = xf3[b][:, (1 - ih) * NI : (2 - ih) * NI]
        xf_aug[C] = 1.0
        in_maps.append(
            {
                "xf": xf_aug.astype(ml_dtypes.bfloat16),
                "xq": np.ascontiguousarray(
                    xr3[b][:, ih * NI : (ih + 1) * NI]
                ).astype(ml_dtypes.bfloat16),
                "cb": cblob.astype(ml_dtypes.bfloat16),
            }
        )

    from concourse.bass_utils import run_bass_kernel_spmd

    res = run_bass_kernel_spmd(nc, in_maps, list(range(NCORES)))

    out = np.empty((B, C, N), np.float32)
    for core in range(NCORES):
        b, ih = core >> 1, core & 1
        out[b][:, ih * NI : (ih + 1) * NI] = res.results[core]["y"]
    return out.reshape(B, C, HH, WW)


# revision 24
# speedup vs baseline: 1.7039x; 1.0129x over previous
"""Trainium2 Bass kernel for the MFPA attention module.

Reference computation (per batch b, with N = H*W = 4096 spatial sites):
    q = Wq @ x_RGB + bq            (CQK=16 channels)
    k = Wk @ x    + bk
    v = Wv @ x    + bv             (C=64 channels)
    energy[i,j] = q_i . k_j
    att = softmax(energy, axis=j)
    out[c,i] = sum_j v[c,j] att[i,j]
    y = lam * out + x

Device strategy (8 NeuronCores): data-parallel over batch (4) x query-row
halves (2).  Each core holds x[b] fully (for K/V and the residual) and its
2048-row query slice, and computes a flash-style streaming softmax so the
4096x4096 energy matrix never leaves PSUM/SBUF.

Host-side weight folding (softmax is shift-invariant, so bk drops out):
    energy[i,j] = (M^T xr_i + bqk) . xf_j    with  M = Wq^T Wk, bqk = Wk^T bq

v2 pipeline notes:
  * exp of the energy is split across BOTH the Scalar (true Exp -> fp8e4)
    and Vector engines (exponent-stuffing: byte = clamp(K8*e + 40) is the
    fp8e4 bit pattern of exp(e)/4, computed by one tensor_scalar).  A
    constant 40/K8 is folded into the energy via the ones row of xf_aug
    (extra qk channel), so the DVE op needs only (mult, max).
  * attention weights p and v are fp8e4; the PV matmul runs DoubleRow
    (256-deep contraction), halving tensor-engine streaming time.
  * the 1/rowsum uses the fast custom-DVE reciprocal; lam and a fp8
    range scale S are folded into wv on the host.
  * residual is taken from the bf16 xf tile (xres input dropped).
  * a short burst of dummy matmuls warms the PE HAM clock-gate while the
    input DMAs stream.
"""

import ml_dtypes
import numpy as np

import concourse.bass as bass
import concourse.mybir as mybir
import concourse.tile as tile_mod
from concourse.vector_clock import ScopedClock

B, C, HH, WW = 4, 64, 64, 64
N = HH * WW          # 4096 spatial sites
NI = N // 2          # query rows per core
CHUNK = 512          # query rows processed per main-loop iteration
NCHUNK = NI // CHUNK
JBLK = 128           # key/value block (PSUM partition dim)
NJP = N // (2 * JBLK)  # 16 j-pair groups (256 keys each)
NCORES = 8
NWARM = 10           # HAM warm-up matmuls

F32 = mybir.dt.float32
F32R = mybir.dt.float32r
BF16 = mybir.dt.bfloat16
F8 = mybir.dt.float8e4
U8 = mybir.dt.uint8
I32 = mybir.dt.int32

K8 = 8.0 / float(np.log(2.0))       # 11.5416  (fp8e4 octave = 8 bytes)
BIAS_B = 40.0                       # byte bias; fp8e4 exp bias is 56 -> p = exp(e)/4
BIAS_ROW = BIAS_B / K8              # energy offset carried by the qk ones-channel
ACT_BIAS = -(BIAS_ROW + float(2.0 * np.log(2.0)))   # Exp(e' + ACT_BIAS) = exp(e)/4
VSCALE = 16.0                       # fp8 range scale on v (cancels in p@v / p@1)
# exp on the vector engine for these j-blocks (14 of 32; DVE also carries
# the bit-trick reciprocal + epilogue muls), scalar takes 18
DVE_BLOCKS = frozenset(j for j in range(1, 32, 2) if j != 15)
RMAGIC = float(0x7EF31000)          # bits(1/x) ~= RMAGIC - bits(x), ~5% max err
NWARM2 = 24


def _patched_drain_and_barrier(self, tick_clock, wait_clock):
    # The walrus build in this container rejects instructions with more than
    # one sync-wait command ("Too many sync wait commands" on the Tile tail
    # drain).  Split the aggregated drain into one drain per semaphore wait.
    nc = self.nc
    drain_inst = nc.sync.drain()
    wait_clock.add_sem_waits(
        drain_inst.ins, ScopedClock({None: tick_clock.global_clock})
    )
    inst = drain_inst.ins
    si = inst.sync_info
    waits = list(si.on_wait or []) if si else []
    if len(waits) > 1:
        si.on_wait = waits[:1]
        for w in waits[1:]:
            extra = nc.sync.drain()
            extra.ins.sync_info = mybir.SyncInfo(on_wait=[w], on_update=[])
    nc.all_engine_barrier()
    popped = nc._tile_sem_poison_stack.pop()
    assert popped is self._sem_poison
    nc.clear_and_free_semaphores(list(self.sems.allocated().values()))
    nc.all_engine_barrier()


tile_mod.TileContext._drain_and_barrier = _patched_drain_and_barrier


def _split_multi_waits(nc):
    # This walrus build accepts at most one sync-wait command per TPB
    # instruction.  Hoist extra waits onto engine NoOps placed just before
    # the instruction (engine executes in order, so semantics are kept).
    for blk in nc.m.functions[0].blocks:
        insts = list(blk.instructions)
        out = []
        changed = False
        for inst in insts:
            si = inst.sync_info
            if si is not None and si.on_wait and len(si.on_wait) > 1:
                waits = list(si.on_wait)
                si.on_wait = waits[-1:]
                for w in waits[:-1]:
                    nop = mybir.InstNoOp(name=nc.get_next_instruction_name())
                    nop.engine = inst.engine
                    nop.sync_info = mybir.SyncInfo(on_wait=[w], on_update=[])
                    out.append(nop)
                changed = True
            out.append(inst)
        if changed:
            blk.instructions = out


def build_bass(split_waits=True):
    nc = bass.Bass()
    xf = nc.declare_dram_parameter("xf", [C + 1, N], BF16, isOutput=False)
    xq = nc.declare_dram_parameter("xq", [C, NI], BF16, isOutput=False)
    # consts blob: cols 0:64 = M, col 64 = zero, col 65 = bias, cols 66:132 = wv_aug
    cb = nc.declare_dram_parameter("cb", [128, 132], BF16, isOutput=False)
    y = nc.declare_dram_parameter("y", [C, NI], F32, isOutput=True)

    with tile_mod.TileContext(nc) as tc:
        with (
            tc.tile_pool(name="singles", bufs=1) as singles,
            tc.tile_pool(name="ppool", bufs=6) as ppool,
            tc.tile_pool(name="ypool", bufs=3) as ypool,
            tc.tile_pool(name="lpool", bufs=2) as lpool,
            tc.tile_pool(name="rpool", bufs=2) as rpool,
            tc.tile_pool(name="ps_et", bufs=5, space="PSUM") as ps_et,
            tc.tile_pool(name="ps_pv", bufs=2, space="PSUM") as ps_pv,
            tc.tile_pool(name="ps_misc", bufs=1, space="PSUM") as ps_misc,
        ):
            # ---- tiny SBUF constants (no DMA) -----------------------------
            # warm-up operand tile: contents are irrelevant (no consumer);
            # a one-column memset just forces allocation
            wsb = singles.tile([128, CHUNK], BF16)
            nc.vector.memset(wsb[:, 0:1], 0.0)
            ones_sb = singles.tile([1, C], F32)
            nc.vector.memset(ones_sb, 1.0)
            actb_sb = singles.tile([128, 1], F32)
            nc.vector.memset(actb_sb, ACT_BIAS)
            # the first ACTIVATE pays the ~2.7us exp table load; fire it on a
            # dummy tile immediately so it overlaps the input DMAs/warm-up
            tblw = singles.tile([128, 1], F32)
            nc.scalar.activation(
                out=tblw, in_=actb_sb,
                func=mybir.ActivationFunctionType.Exp,
            )

            # ---- input DMAs: few triggers, spread across the three DMA-
            # capable queues (sync / gpsimd / scalar) for parallel rings
            xq_sb = singles.tile([C, NI], BF16)
            nc.scalar.dma_start(out=xq_sb, in_=xq[:, :])
            cb_sb = singles.tile([128, 132], BF16)
            nc.gpsimd.dma_start(out=cb_sb, in_=cb[:, :])
            # K=128 contraction everywhere (the fast weight-load path needs
            # full-height weights); rows C+1..127 are zero on the qk side.
            xf_sb = singles.tile([128, N], BF16)
            nc.vector.memset(xf_sb[64:128, :], 0.0)
            NQ = N // 4
            nc.sync.dma_start(out=xf_sb[0 : C + 1, 0:NQ], in_=xf[:, 0:NQ])
            nc.gpsimd.dma_start(
                out=xf_sb[0 : C + 1, NQ : 2 * NQ], in_=xf[:, NQ : 2 * NQ]
            )
            nc.sync.dma_start(
                out=xf_sb[0 : C + 1, 2 * NQ : 3 * NQ], in_=xf[:, 2 * NQ : 3 * NQ]
            )
            nc.scalar.dma_start(
                out=xf_sb[0 : C + 1, 3 * NQ : N], in_=xf[:, 3 * NQ : N]
            )
            # xf arrives column-permuted so this core's own query half sits
            # at columns 0:NI (softmax is j-permutation invariant); row C is
            # all-ones: it feeds the V bias row, the qk energy-bias channel
            # and the fp8 rowsum column.

            # ---- HAM warm-up: a gapless accumulating matmul burst latches
            # the PE clock-gate to full speed while the input DMAs stream
            wp = ps_misc.tile([128, CHUNK], F32, tag="misc")
            for k in range(NWARM2):
                nc.tensor.matmul(
                    out=wp, lhsT=wsb[:, 0:128], rhs=wsb,
                    start=(k == 0), stop=(k == NWARM2 - 1),
                )

            bqk_sb = singles.tile([C + 1, 1], F32)
            nc.vector.tensor_copy(bqk_sb, cb_sb[0 : C + 1, 65:66])

            # ---- Q.K preparation -----------------------------------------
            # qs row C is 0 (M blob column 64..65 region row C is zero), and
            # the bias vector carries BIAS_ROW there, so the Identity
            # activation fills the whole (C+1)-row qk tile in one op.
            # Only chunk 0 is prepared up front; chunk ic+1's qk is produced
            # inside chunk ic's block stream so the PE never idles on it.
            qk_sbs = [
                singles.tile([128, CHUNK], BF16, name=f"qk_sb{ic}")
                for ic in range(NCHUNK)
            ]
            for t in qk_sbs:
                nc.vector.memset(t[64:128, :], 0.0)

            def emit_qk_prep(ic):
                isl = slice(ic * CHUNK, (ic + 1) * CHUNK)
                qs = ps_misc.tile([C + 1, CHUNK], F32, tag="misc")
                nc.tensor.matmul(
                    out=qs, lhsT=cb_sb[0:C, 0:65], rhs=xq_sb[:, isl],
                    start=True, stop=True,
                )
                nc.scalar.activation(
                    out=qk_sbs[ic][0 : C + 1, :], in_=qs,
                    func=mybir.ActivationFunctionType.Identity,
                    bias=bqk_sb[:, 0:1], scale=1.0,
                )

            emit_qk_prep(0)

            # ---- V preparation: fp8 v_pair[jp, pair, t, c] ----------------
            # pair-dim byte step must be 16-aligned for DoubleRow ldweights -> pad 66 to 80
            v_sb = singles.tile([JBLK, NJP, 2, 80], F8)

            def emit_vprep(m4):
                vp = ps_misc.tile([JBLK, 2, 2, 66], F32, tag="misc")
                for k in range(4):
                    jb = 4 * m4 + k
                    nc.tensor.matmul(
                        out=vp[:, k // 2, k % 2, :],
                        lhsT=xf_sb[:, jb * JBLK : (jb + 1) * JBLK],
                        rhs=cb_sb[0:128, 66:132],
                        start=True, stop=True,
                    )
                nc.scalar.activation(
                    out=v_sb[:, 2 * m4 : 2 * m4 + 2, :, 0:66], in_=vp,
                    func=mybir.ActivationFunctionType.Copy,
                )

            for m4 in range(3):
                emit_vprep(m4)

            # ---- main loop over query chunks ------------------------------
            # Per chunk: 32 j-block energy matmuls -> exp (split ACT/DVE)
            # -> DoubleRow PV accumulation, software-pipelined so the PE
            # never waits: PV of pair jp issues ~2 j-blocks after its exp,
            # and the previous chunk's epilogue tail (which trails the slow
            # reciprocal) is emitted a few groups into the next chunk.
            tail_ops = []
            pvq = []          # deferred cross-chunk PV tail + reciprocal
            for ic in range(NCHUNK):
                isl = slice(ic * CHUNK, (ic + 1) * CHUNK)
                qk_sb = qk_sbs[ic]
                pv = ps_pv.tile([66, CHUNK], F32)
                p_t = None
                for jb in range(2 * NJP):
                    et = ps_et.tile([JBLK, CHUNK], F32)
                    nc.tensor.matmul(
                        out=et,
                        lhsT=xf_sb[:, jb * JBLK : (jb + 1) * JBLK],
                        rhs=qk_sb,
                        start=True, stop=True,
                    )
                    if jb % 2 == 0:
                        p_t = ppool.tile([JBLK, 2, CHUNK], F8)
                        p_ts = [p_t] if jb == 0 else p_ts + [p_t]
                    if jb in DVE_BLOCKS:
                        # byte = clamp(round(K8*e + 40)) == fp8e4(exp(e)/4)
                        nc.vector.tensor_scalar(
                            out=p_t[:, jb % 2, :].bitcast(U8), in0=et[:, :],
                            scalar1=K8, scalar2=0.0,
                            op0=mybir.AluOpType.mult, op1=mybir.AluOpType.max,
                        )
                    else:
                        nc.scalar.activation(
                            out=p_t[:, jb % 2, :], in_=et[:, :],
                            func=mybir.ActivationFunctionType.Exp,
                            bias=actb_sb[:, 0:1], scale=1.0,
                        )
                    if jb in (1, 3) and pvq:
                        pvq.pop(0)()
                    if jb == 8 and tail_ops:
                        tail_ops.pop(0)()
                    if jb == 12 and ic + 1 < NCHUNK:
                        emit_qk_prep(ic + 1)
                    if ic == 0 and jb in (2, 6, 10, 14, 18):
                        emit_vprep(3 + (jb - 2) // 4)
                    if jb >= 5 and (jb - 5) % 2 == 0:
                        jp = (jb - 5) // 2
                        nc.tensor.matmul(
                            out=pv[:, :], lhsT=v_sb[:, jp, :, 0:66],
                            rhs=p_ts[jp][:, :, :],
                            start=(jp == 0), stop=False,
                            perf_mode=mybir.MatmulPerfMode.DoubleRow,
                        )

                def make_pv_tail(pv=pv, p_ts=p_ts, jp=NJP - 2):
                    def emit():
                        nc.tensor.matmul(
                            out=pv[:, :], lhsT=v_sb[:, jp, :, 0:66],
                            rhs=p_ts[jp][:, :, :],
                            start=False, stop=(jp == NJP - 1),
                            perf_mode=mybir.MatmulPerfMode.DoubleRow,
                        )
                    return emit

                def make_recip(pv=pv, ic=ic, isl=isl):
                    def emit():
                        make_pv_tail(pv=pv, jp=NJP - 1)()
                        # exponent-flip bit-trick reciprocal of the rowsum
                        r0 = rpool.tile([1, CHUNK], I32)
                        nc.vector.tensor_scalar(
                            out=r0, in0=pv[64:65, :].bitcast(I32),
                            scalar1=-1.0, scalar2=RMAGIC,
                            op0=mybir.AluOpType.mult, op1=mybir.AluOpType.add,
                        )
                        tail_ops.append(make_tail(pv, r0, isl, ic))
                    return emit

                def make_tail(pv, r0, isl, ic):
                    def emit():
                        # true-fp32 broadcast matmul reads the reciprocal
                        # bits directly (no conversion op on any engine)
                        lrb = ps_misc.tile([C, CHUNK], F32, tag="misc")
                        nc.tensor.matmul(
                            out=lrb, lhsT=ones_sb[:, :],
                            rhs=r0[:, :].bitcast(F32),
                            start=True, stop=True,
                        )
                        lrb_sb = lpool.tile([C, CHUNK], F32)
                        nc.scalar.copy(lrb_sb, lrb)
                        y_t = ypool.tile([C, CHUNK], F32)
                        nc.vector.tensor_tensor(
                            out=y_t, in0=pv[0:C, :], in1=lrb_sb,
                            op=mybir.AluOpType.mult,
                        )
                        nc.vector.tensor_tensor(
                            out=y_t, in0=y_t, in1=xf_sb[0:C, isl],
                            op=mybir.AluOpType.add,
                        )
                        eng = nc.sync if ic % 2 == 0 else nc.gpsimd
                        eng.dma_start(out=y[:, isl], in_=y_t)
                    return emit

                pvq = [make_pv_tail(), make_recip()]
            for fn in pvq:
                fn()
            for fn in tail_ops:
                fn()

    if split_waits:
        _split_multi_waits(nc)
    return nc


_CACHE = {}


def kernel(**inputs):
    x = np.ascontiguousarray(np.asarray(inputs["x"], dtype=np.float32))
    x_RGB = np.ascontiguousarray(np.asarray(inputs["x_RGB"], dtype=np.float32))
    Wq = np.asarray(inputs["Wq"], dtype=np.float32)
    bq = np.asarray(inputs["bq"], dtype=np.float32)
    Wk = np.asarray(inputs["Wk"], dtype=np.float32)
    Wv = np.asarray(inputs["Wv"], dtype=np.float32)
    bv = np.asarray(inputs["bv"], dtype=np.float32)
    lam = np.asarray(inputs["lam"], dtype=np.float32)

    M = (Wq.T.astype(np.float64) @ Wk.astype(np.float64)).astype(np.float32)
    bqk = (Wk.T.astype(np.float64) @ bq.astype(np.float64)).astype(np.float32)

    ls = float(lam.reshape(-1)[0]) * VSCALE
    wv_aug = np.zeros((C + 1, 66), np.float32)
    wv_aug[:C, :C] = Wv.T * ls
    wv_aug[C, :C] = bv * ls
    wv_aug[C, 64] = VSCALE

    cblob = np.zeros((128, 132), np.float32)
    cblob[0:C, 0:64] = M
    cblob[0:C, 65] = bqk
    cblob[C, 65] = BIAS_ROW
    cblob[0 : C + 1, 66:132] = wv_aug

    xf3 = x.reshape(B, C, N)
    xr3 = x_RGB.reshape(B, C, N)

    if "nc" not in _CACHE:
        _CACHE["nc"] = build_bass()
    nc = _CACHE["nc"]

    in_maps = []
    for core in range(NCORES):
        b, ih = core >> 1, core & 1
        xf_aug = np.empty((C + 1, N), np.float32)
        # own query half first (static residual slice), other half after
        xf_aug[:C, :NI] = xf3[b][:, ih * NI : (ih + 1) * NI]
        xf_aug[:C, NI:] = xf3[b][:, (1 - ih) * NI : (2 - ih) * NI]
        xf_aug[C] = 1.0
        in_maps.append(
            {
                "xf": xf_aug.astype(ml_dtypes.bfloat16),
                "xq": np.ascontiguousarray(
                    xr3[b][:, ih * NI : (ih + 1) * NI]
                ).astype(ml_dtypes.bfloat16),
                "cb": cblob.astype(ml_dtypes.bfloat16),
            }
        )

    from concourse.bass_utils import run_bass_kernel_spmd

    res = run_bass_kernel_spmd(nc, in_maps, list(range(NCORES)))

    out = np.empty((B, C, N), np.float32)
    for core in range(NCORES):
        b, ih = core >> 1, core & 1
        out[b][:, ih * NI : (ih + 1) * NI] = res.results[core]["y"]
    return out.reshape(B, C, HH, WW)
